# revision 2
# baseline (speedup 1.0000x reference)
"""Causal self-attention with RoPE, tensor-parallel over heads on 8 trn2 cores.

Reference computation (B=1, T=4096, C=1024, h=16, d=64, fp32):
    q/k/v = x @ W{q,k,v}^T ; rope(q), rope(k) ; causal softmax(q k^T / 8) v ; @ Wo^T

Sharding: 2 heads per core (tensor parallel). Each core reads the full x
(transposed + bf16 on host) and its slice of Wq/Wk/Wv (column-parallel) and
Wo (row-parallel). Cores emit partial o-projections; the host sums them.

Device-side layout choices (v2):
  - qT/kT [dhead(=128 both heads) x T] with the head dim de-interleaved
    (rope real parts in partitions 0-31 / 64-95, imag in 32-63 / 96-127) so
    rope's pair-swap is a partition-block swap done by 4 small DMAs.
  - scores are computed transposed: sT[j, i] = sum_d kT[d,j] qT[d,i].
  - exp runs on ACT (hw Exp) or DVE (bf16 Schraudolph: bitcast(int16(
    x*A + B)), ~1% softmax-weight ripple that largely cancels between the
    softmax numerator and denominator); diagonal blocks fold the causal
    mask into a DVE scalar_tensor_tensor with a +B / -20000 tile (masked
    lanes land at ~1e-37 ~ 0), or run ACT exp + 0/1 mask multiply,
    restricted to the valid column range.  A static schedule balances the
    two engines.
  - att@v is computed with att as the *stationary* operand ([128k x 128q]
    blocks) and the ones-augmented v [128k x 65] moving, 65 PE cycles per
    block; the 4 query sub-blocks of a head accumulate into one psum bank
    as a single lazily-zeroed accumulation group (row 64 = softmax
    denominator); fully-masked diagonal sub-blocks are skipped.
  - normalize: per-query reciprocal of the denominators + scaled
    psum->sbuf copies per (qs, head); one batched XBAR dma-transpose per
    window rebuilds yT [dd x tok] for the o-projection (bf16), whose
    output rows go out in 256-row batched DMAs.
"""

import numpy as np
import ml_dtypes

bf16 = ml_dtypes.bfloat16

T, C, H, D = 4096, 1024, 16, 64
NCORES = 8
HPC = H // NCORES          # heads per core
DD = HPC * D               # per-core qkv features (=128)
P = 128

A0 = 128.0 / np.log(2.0)   # Schraudolph bf16: e^x ~ bitcast(int16(x*A0 + B0))
B0 = 127.0 * 128.0 - 7.0
SCALE = 0.125
MASKED = -20000.0

_nc_cache = {}


def _exp_schedule(nw):
    """(iw, jc) -> 'a' | 'd' (interior) | 'am' | 'dm' (diagonal)."""
    import os
    sched = {}
    k = dk = 0
    pat = tuple(os.environ.get("EXP_PAT", "adad"))
    dpat = tuple((os.environ.get("DIAG_PAT", "damd") + "m")[i] + "m"
                 for i in range(4)) if False else None
    dpat = tuple(c + "m" for c in os.environ.get("DIAG_PAT", "aada"))
    if os.environ.get("EXP_ALL") == "a":
        pat, dpat = ("a",), ("am",)
    elif os.environ.get("EXP_ALL") == "d":
        pat, dpat = ("d",), ("dm",)
    for iw in range(nw):
        njc = 4 * iw + 4
        for jc in range(njc):
            if jc >= 4 * iw:
                sched[(iw, jc)] = dpat[(jc - 4 * iw) % len(dpat)]
                dk += 1
            else:
                sched[(iw, jc)] = pat[k % len(pat)]
                k += 1
    return sched


def _build_nc(t=T):
    import concourse.bass as bass
    import concourse.tile as tile
    import concourse.mybir as mybir
    from concourse import bacc

    f32 = mybir.dt.float32
    b16 = mybir.dt.bfloat16
    i16 = mybir.dt.int16
    MUL = mybir.AluOpType.mult
    ADD = mybir.AluOpType.add
    EXP = mybir.ActivationFunctionType.Exp
    CPY = mybir.ActivationFunctionType.Copy

    nt = t // 512            # qkv t-chunks
    nw = t // 512            # attention query windows
    njb = t // P             # key blocks
    AS = SCALE * A0

    sched = _exp_schedule(nw)

    nc = bacc.Bacc("TRN2")

    xt_d = nc.dram_tensor("xt", [C, t], b16, kind="ExternalInput")
    # weights host-prepacked to [P, C//P, DD]: one contiguous 2KB run per
    # partition (full-rate DMA descriptors)
    wq_d = nc.dram_tensor("wq", [P, C // P, DD], b16, kind="ExternalInput")
    wk_d = nc.dram_tensor("wk", [P, C // P, DD], b16, kind="ExternalInput")
    wv_d = nc.dram_tensor("wv", [P, C // P, DD], b16, kind="ExternalInput")
    wo_d = nc.dram_tensor("wo", [DD, C], b16, kind="ExternalInput")
    cos_d = nc.dram_tensor("cosb", [P, t], b16, kind="ExternalInput")
    sin_d = nc.dram_tensor("sinb", [P, t], b16, kind="ExternalInput")
    msk_d = nc.dram_tensor("mask4", [P, 4, 512], b16, kind="ExternalInput")
    mskb_d = nc.dram_tensor("maskb", [P, 4, 512], f32, kind="ExternalInput")
    out_d = nc.dram_tensor("opart", [t, C], b16, kind="ExternalOutput")

    with tile.TileContext(nc) as tc:
        with (
            tc.tile_pool(name="const", bufs=1) as constp,
            tc.tile_pool(name="xload", bufs=3) as xload,
            tc.tile_pool(name="rope", bufs=3) as ropep,
            tc.tile_pool(name="att", bufs=4) as attp,
            tc.tile_pool(name="small", bufs=4) as smallp,
            tc.tile_pool(name="ps", bufs=1, space="PSUM") as psp,
        ):
            # ---- constants / persistent tensors. DMA issue order chosen so
            # the first qkv matmuls (wq + x chunk 0) start ASAP.
            xt_pre = {}

            def load_xt(tch):
                tsl = slice(tch * 512, (tch + 1) * 512)
                xt = xload.tile([P, C // P, 512], b16, name="xt", tag="xt")
                nc.sync.dma_start(
                    xt, xt_d[:].rearrange("(co p) t -> p co t", p=P)[:, :, tsl]
                )
                xt_pre[tch] = xt

            wq_sb = constp.tile([P, C // P, DD], b16)
            nc.sync.dma_start(wq_sb, wq_d[:])
            # x chunk 0 in four slices so the first qkv matmuls start sooner
            xt0 = xload.tile([P, C // P, 512], b16, name="xt0")
            xt_view = xt_d[:].rearrange("(co p) t -> p co t", p=P)
            for c4 in range(4):
                nc.sync.dma_start(xt0[:, 2 * c4:2 * c4 + 2],
                                  xt_view[:, 2 * c4:2 * c4 + 2, 0:512])
            xt_pre[0] = xt0
            wk_sb = constp.tile([P, C // P, DD], b16)
            nc.sync.dma_start(wk_sb, wk_d[:])
            load_xt(1)
            wv_sb = constp.tile([P, C // P, DD], b16)
            nc.sync.dma_start(wv_sb, wv_d[:])
            # rope tables: first two chunks' columns now, tails late
            cos_sb = constp.tile([P, t], b16)
            nc.sync.dma_start(cos_sb[:, 0:1024], cos_d[:, 0:1024])
            sin_sb = constp.tile([P, t], b16)
            nc.sync.dma_start(sin_sb[:, 0:1024], sin_d[:, 0:1024])
            # the first window's diagonal needs maskb[0] immediately
            mskb_sb = constp.tile([P, 4, 512], f32)
            nc.sync.dma_start(mskb_sb[:, 0:1], mskb_d[:, 0:1])
            msk_sb = constp.tile([P, 4, 512], b16)
            nc.sync.dma_start(msk_sb[:, 0:1], msk_d[:, 0:1])
            wo_sb = constp.tile([DD, C], b16)

            def late_consts():
                nc.sync.dma_start(mskb_sb[:, 1:4], mskb_d[:, 1:4])
                nc.sync.dma_start(msk_sb[:, 1:4], msk_d[:, 1:4])
                if t > 1024:
                    nc.sync.dma_start(cos_sb[:, 1024:], cos_d[:, 1024:])
                    nc.sync.dma_start(sin_sb[:, 1024:], sin_d[:, 1024:])
                nc.sync.dma_start(wo_sb, wo_d[:])

            qT = constp.tile([P, t], b16)   # rope'd q, both heads
            kT = constp.tile([P, t], b16)
            # v in natural layout per 128-block, +ones cols at 64 and 129
            vaug = constp.tile([P, njb, 2 * D + 2], b16)
            nc.vector.memset(vaug[:, :, D], 1.0)
            nc.vector.memset(vaug[:, :, 2 * D + 1], 1.0)

            # PSUM budget (8 banks): pss2 4KB x3 (scores, triple-buffered;
            # qkv and o-proj psums borrow half-slots from the same rotation)
            # + psy0/psy1 2KB x1 (att@v + denominator accumulators).
            def scratch():
                return psp.tile([P, 512], f32, tag="scratch", bufs=2,
                                name="scr")

            ob_k = {"k": 0}

            def drain_copy(dst, src_):
                """psum->sbuf copy alternating ACT/DVE."""
                ob_k["k"] += 1
                import os as _o4
                pat4 = _o4.environ.get("DRAIN", "ad")
                if pat4[ob_k["k"] % len(pat4)] == "a":
                    nc.scalar.copy(dst, src_)
                else:
                    nc.vector.tensor_copy(dst, src_)

            def qkv_parts(tch, cp):
                """Six micro-closures (~0.85us of PE each) spread through an
                attention window's PE slack. cp = engine for the psum->sbuf
                copies (ACT before the windows saturate it, DVE after)."""
                tsl = slice(tch * 512, (tch + 1) * 512)
                st = {}
                import os as _o3
                reng = nc.gpsimd if (
                    _o3.environ.get("GPSROPE", "1") == "1" and tch >= int(_o3.environ.get("GPSROPE_MIN", "0"))) \
                    else nc.vector

                def copy(dst, src_):
                    if cp is nc.scalar:
                        nc.scalar.copy(dst, src_)
                    else:
                        cp.tensor_copy(dst, src_)

                def mm_half(name, w_sb, half):
                    if name == "q" and half == 0:
                        st["xt"] = xt_pre.pop(tch)
                        st["q"] = scratch()
                    if name == "k" and half == 0:
                        if tch + 2 < nt and tch + 2 not in xt_pre:
                            load_xt(tch + 2)
                    if half == 0 and name != "q":
                        st[name] = scratch()
                    ps = st[name]
                    for ci in range(4 * half, 4 * half + 4):
                        nc.tensor.matmul(
                            ps, w_sb[:, ci], st["xt"][:, ci],
                            start=(ci == 0), stop=(ci == C // P - 1),
                        )

                def q1():
                    mm_half("q", wq_sb, 0)

                def q2():
                    mm_half("q", wq_sb, 1)
                    # q and k share one [128, 2, 512] tile; the rope
                    # pair-swap is partition-block DMAs issued right after
                    # each tensor's psum copy so the swap latency overlaps
                    # the other tensor's matmuls
                    st["qf2"] = ropep.tile([P, 2, 512], b16, tag="qf2",
                                           name="qf2")
                    st["sw2"] = ropep.tile([P, 2, 512], b16, tag="sw2",
                                           name="sw2")
                    copy(st["qf2"][:, 0], st["q"])
                    import os as _o2
                    if _o2.environ.get("EARLYSWAP") == "1":
                        qf2, sw2 = st["qf2"], st["sw2"]
                        nc.sync.dma_start(sw2[0:32, 0], qf2[32:64, 0])
                        nc.sync.dma_start(sw2[32:64, 0], qf2[0:32, 0])
                        nc.sync.dma_start(sw2[64:96, 0], qf2[96:128, 0])
                        nc.sync.dma_start(sw2[96:128, 0], qf2[64:96, 0])

                def k1():
                    mm_half("k", wk_sb, 0)

                def k2():
                    mm_half("k", wk_sb, 1)
                    qf2 = st["qf2"]
                    sw2 = st["sw2"]
                    copy(qf2[:, 1], st["k"])
                    import os as _o2
                    if _o2.environ.get("EARLYSWAP") == "1":
                        nc.sync.dma_start(sw2[0:32, 1], qf2[32:64, 1])
                        nc.sync.dma_start(sw2[32:64, 1], qf2[0:32, 1])
                        nc.sync.dma_start(sw2[64:96, 1], qf2[96:128, 1])
                        nc.sync.dma_start(sw2[96:128, 1], qf2[64:96, 1])
                    else:
                        nc.sync.dma_start(sw2[0:32], qf2[32:64])
                        nc.sync.dma_start(sw2[32:64], qf2[0:32])
                        nc.sync.dma_start(sw2[64:96], qf2[96:128])
                        nc.sync.dma_start(sw2[96:128], qf2[64:96])
                    for i, name in enumerate(("q", "k")):
                        tl = ropep.tile([P, 512], b16, tag=f"t1_{name}",
                                        name="t1")
                        reng.tensor_tensor(tl, qf2[:, i],
                                           cos_sb[:, tsl], MUL)
                        st[f"t1{name}"] = tl

                def v1():
                    t2q = ropep.tile([P, 512], b16, tag="t2_q", name="t2")
                    reng.tensor_tensor(t2q, st["sw2"][:, 0],
                                       sin_sb[:, tsl], MUL)
                    reng.tensor_add(qT[:, tsl], st["t1q"], t2q)
                    mm_half("v", wv_sb, 0)

                def v2():
                    mm_half("v", wv_sb, 1)
                    t2k = ropep.tile([P, 512], b16, tag="t2_k", name="t2")
                    reng.tensor_tensor(t2k, st["sw2"][:, 1],
                                       sin_sb[:, tsl], MUL)
                    reng.tensor_add(kT[:, tsl], st["t1k"], t2k)
                    # v: psum->sbuf copy, XBAR dma-transpose to dense [t, d]
                    # blocks, two narrow gpsimd copies around the ones column
                    vt = ropep.tile([P, 512], b16, tag="vt", name="vt")
                    copy(vt, st["v"])
                    dense = ropep.tile([P, 4, P], b16, tag="vdense",
                                       name="vdense")
                    nc.sync.dma_start_transpose(dense, vt[:])
                    g0 = tch * 4
                    nc.gpsimd.tensor_copy(vaug[:, g0:g0 + 4, 0:D],
                                          dense[:, :, 0:D])
                    nc.gpsimd.tensor_copy(vaug[:, g0:g0 + 4, D + 1:2 * D + 1],
                                          dense[:, :, D:2 * D])

                return [q1, q2, k1, k2, v1, v2]

            def qkv_chunk(tch):
                for f in qkv_parts(tch, nc.scalar):
                    f()

            def oproj_fillers(iw, yT, tail=False):
                """o-projection for two 256-row blocks, one batched output
                DMA each. Stationary yT blocks come from the window's XBAR
                transpose."""
                outs = []
                for pi in range(2):
                    def one(pi=pi):
                        ob4 = attp.tile([P, 2, 2, 512], b16, tag="ob4",
                                        bufs=2, name="ob4")
                        for j in range(2):
                            g = 2 * pi + j
                            for mc in range(2):
                                pso = scratch()
                                nc.tensor.matmul(
                                    pso, yT[:, g],
                                    wo_sb[:, mc * 512:(mc + 1) * 512],
                                    start=True, stop=True,
                                )
                                drain_copy(ob4[:, j, mc], pso)
                        tb0 = 4 * iw + 2 * pi
                        dst = out_d[tb0 * P:(tb0 + 2) * P, :].rearrange(
                            "(j p) (mc i) -> p j mc i", j=2, mc=2)
                        nc.sync.dma_start(dst, ob4)
                    outs.append(one)
                return outs

            def win512(iw, parts=(), oproj=(), last=False, head=()):
                """512-wide attention window over i in [512iw, 512iw+512),
                one-jc software-pipelined. Fillers run in the slack of the
                jc loop: qkv chunk micro-parts from jc>=1, the o-projection
                of window iw-2 from jc>=3."""
                isl = slice(iw * 512, (iw + 1) * 512)
                njc = 4 * iw + 4
                parts = list(parts)
                oproj = list(oproj)
                psy = [psp.tile([P, 4, P], f32, tag=f"psy{h}",
                                name="psy") for h in range(HPC)]

                def score_exp(jc, isl_, oiw):
                    eng = sched[(oiw, jc)]
                    m = jc - 4 * oiw if jc >= 4 * oiw else None
                    import os as _o
                    lo = m * P if (m is not None and _o.environ.get(
                        "DIAG_SCORE") == "1") else 0
                    jsl = slice(jc * P, (jc + 1) * P)
                    ps2 = psp.tile([P, 2, 512], f32, tag="pss2", bufs=2,
                                   name="ps2")
                    for h in range(HPC):
                        hb = D * h
                        nc.tensor.matmul(
                            ps2[:, h, lo:],
                            kT[hb:hb + D, jsl],
                            qT[hb:hb + D, isl_.start + lo:isl_.stop],
                            start=True, stop=True)
                    att2 = attp.tile([P, 2, 512], b16, tag="att2",
                                     name="att2", bufs=10)
                    if m is None:
                        if _o.environ.get("EXPSPLIT") == "1":
                            nc.scalar.activation(att2[:, 0], ps2[:, 0], EXP,
                                                 scale=SCALE)
                            nc.vector.tensor_scalar(
                                att2[:, 1].bitcast(i16), ps2[:, 1],
                                AS, B0, MUL, ADD)
                        elif eng == "a":
                            nc.scalar.activation(att2, ps2, EXP, scale=SCALE)
                        else:
                            nc.vector.tensor_scalar(att2[:].bitcast(i16),
                                                    ps2, AS, B0, MUL, ADD)
                    else:
                        lo = m * P
                        if eng == "am":
                            nc.scalar.activation(att2[:, :, lo:],
                                                 ps2[:, :, lo:], EXP,
                                                 scale=SCALE)
                            for h in range(HPC):
                                nc.vector.tensor_tensor(
                                    att2[:, h, lo:], att2[:, h, lo:],
                                    msk_sb[:, m, lo:], MUL)
                        else:
                            for h in range(HPC):
                                nc.vector.scalar_tensor_tensor(
                                    att2[:, h, lo:].bitcast(i16),
                                    ps2[:, h, lo:], AS,
                                    mskb_sb[:, m, lo:], MUL, ADD)
                    return att2, m

                def emit_attv(p):
                    jc0, a2, m0 = p
                    for h in range(HPC):
                        va = vaug[:, jc0, 65 * h:65 * h + 65]
                        for qs in range(4):
                            if m0 is not None and qs < m0:
                                continue
                            nc.tensor.matmul(
                                psy[h][:, qs, 0:65],
                                a2[:, h, qs * P:(qs + 1) * P], va,
                                start=(jc0 == 0 and qs == 0),
                                stop=(jc0 == njc - 1 and qs == 3))

                pend = list(head)
                for jc in range(len(head), njc):
                    att2, m = score_exp(jc, isl, iw)
                    if len(pend) == int(__import__("os").environ.get(
                            "PEND", "2")):
                        emit_attv(pend.pop(0))
                    pend.append((jc, att2, m))
                    if jc < njc - 3:
                        if jc >= 1 and parts:
                            parts.pop(0)()
                        elif jc >= 3 and oproj and (
                                jc % 2 == 1
                                or 3 * len(oproj) > njc - 3 - jc):
                            oproj.pop(0)()
                nxt = []
                if not last and iw > 0:
                    # pre-compute the next window's first score/exp blocks
                    # so PE has work while this window's tail drains
                    isl_n = slice((iw + 1) * 512, (iw + 2) * 512)
                    for jc_n in range(3):
                        a2n, mn = score_exp(jc_n, isl_n, iw + 1)
                        nxt.append((jc_n, a2n, mn))
                for p in pend:
                    emit_attv(p)
                recs = []
                for h in range(HPC):
                    rec = smallp.tile([P, 4], f32, tag=f"rec{h}", bufs=3,
                                      name="rec")
                    nc.vector.reciprocal(rec, psy[h][:, :, 64])
                    recs.append(rec)
                yT = ropep.tile([P, 4, P], b16, tag="yt", bufs=3, name="yT")

                def norm_half(q0):
                    def f():
                        ysb = attp.tile([P, 2, P], b16, tag=f"ysb{q0}",
                                        bufs=2, name="ysb")
                        for qs in (q0, q0 + 1):
                            for h in range(HPC):
                                dst = ysb[:, qs - q0, 64 * h:64 * h + 64]
                                srcp = psy[h][:, qs, 0:64]
                                if (qs + h) % 2 == 0:
                                    nc.scalar.activation(
                                        dst, srcp, CPY,
                                        scale=recs[h][:, qs:qs + 1])
                                else:
                                    nc.vector.tensor_scalar(
                                        dst, srcp, recs[h][:, qs:qs + 1],
                                        None, MUL)
                        nc.sync.dma_start_transpose(
                            yT[:, q0:q0 + 2, :],
                            ysb[:].rearrange("p q d -> p (q d)"))
                    return f

                import os
                if os.environ.get("OLD_NORM") == "1":
                    ysb = attp.tile([P, 4, P], b16, tag="ysb0", bufs=2,
                                    name="ysb")
                    for qs in range(4):
                        for h in range(HPC):
                            dst = ysb[:, qs, 64 * h:64 * h + 64]
                            srcp = psy[h][:, qs, 0:64]
                            if (qs + h) % 2 == 0:
                                nc.scalar.activation(
                                    dst, srcp, CPY,
                                    scale=recs[h][:, qs:qs + 1])
                            else:
                                nc.vector.tensor_scalar(
                                    dst, srcp, recs[h][:, qs:qs + 1],
                                    None, MUL)
                    nc.sync.dma_start_transpose(
                        yT, ysb[:].rearrange("p q d -> p (q d)"))
                    norms = []
                else:
                    norms = [norm_half(0), norm_half(2)]
                if norms and (last or os.environ.get("NO_DEFER_NORM") == "1"):
                    for f in norms:
                        f()
                    norms = []
                for f in parts + oproj:   # leftovers
                    f()
                return oproj_fillers(iw, yT, tail=last), nxt, norms

            qkv_chunk(0)
            c1 = qkv_parts(1, nc.scalar)
            for f in c1[:4]:
                f()
            late_consts()

            pend_ops = {}
            hd = ()
            norms = []
            for iw in range(nw):
                if iw == 0:
                    parts = c1[4:] + (qkv_parts(2, nc.scalar)
                                      if nt > 2 else [])
                else:
                    tch = iw + 2
                    import os as _o5
                    _cpm = int(_o5.environ.get("CPMAX", "4"))
                    parts = qkv_parts(
                        tch, nc.scalar if tch <= _cpm else nc.vector) \
                        if tch < nt else []
                if iw == nw - 2:
                    oproj = pend_ops.pop(iw - 2, []) + pend_ops.pop(iw - 1,
                                                                    [])
                elif iw == nw - 1:
                    oproj = pend_ops.pop(iw - 1, [])
                else:
                    oproj = pend_ops.pop(iw - 2, [])
                ops, hd, norms = win512(iw, parts=norms + parts,
                                        oproj=oproj,
                                        last=(iw == nw - 1), head=hd)
                pend_ops[iw] = ops
            for f in pend_ops.pop(nw - 1):
                f()

    nc.compile()
    return nc


def _perm_deinterleave():
    """Row permutation for Wq/Wk: per head, even rows then odd rows."""
    perm = []
    for h in range(H):
        base = h * D
        perm += [base + 2 * k for k in range(D // 2)]
        perm += [base + 2 * k + 1 for k in range(D // 2)]
    return np.array(perm)


def make_core_inputs(x, freqs_cos, freqs_sin, Wq, Wk, Wv, Wo, t=T):
    """Host-side sharding/layout prep. Returns per-core input dicts."""
    x = np.asarray(x, np.float32).reshape(t, C)
    fc = np.asarray(freqs_cos, np.float32)
    fs = np.asarray(freqs_sin, np.float32)
    Wq = np.asarray(Wq, np.float32)
    Wk = np.asarray(Wk, np.float32)
    Wv = np.asarray(Wv, np.float32)
    Wo = np.asarray(Wo, np.float32)

    xt = np.ascontiguousarray(x.T).astype(bf16)                  # [C, t]
    perm = _perm_deinterleave()
    Wq_p, Wk_p = Wq[perm], Wk[perm]

    # rope factor tables in the de-interleaved [dd, t] layout
    kidx = np.arange(P) % 32
    sgn = np.where((np.arange(P) // 32) % 2 == 0, -1.0, 1.0).astype(np.float32)
    cosb = fc.T[kidx].astype(bf16)                               # [128, t]
    sinb = (fs.T[kidx] * sgn[:, None]).astype(bf16)

    # diagonal-tile causal masks: valid iff 128*m + j <= i
    jj = np.arange(P)[:, None, None]
    mm = np.arange(4)[None, :, None]
    ii = np.arange(512)[None, None, :]
    valid = (P * mm + jj) <= ii
    mask4 = valid.astype(bf16)
    maskb = np.where(valid, np.float32(B0), np.float32(MASKED))

    def pack_w(w):
        # [C, DD] -> [P, C//P, DD]: one contiguous 2KB run per partition
        return np.ascontiguousarray(
            w.T.reshape(C // P, P, DD).transpose(1, 0, 2)).astype(bf16)

    in_maps = []
    for c in range(NCORES):
        rows = slice(c * DD, (c + 1) * DD)
        in_maps.append({
            "xt": xt,
            "wq": pack_w(Wq_p[rows]),
            "wk": pack_w(Wk_p[rows]),
            "wv": pack_w(Wv[rows]),
            "wo": np.ascontiguousarray(Wo[:, rows].T).astype(bf16),
            "cosb": cosb,
            "sinb": sinb,
            "mask4": mask4,
            "maskb": maskb,
        })
    return in_maps


def run(inputs, trace=False):
    """Compile once, run on 8 cores, host-sum partials."""
    import sys
    if "/opt/trn_rl_repo" not in sys.path:
        sys.path.insert(0, "/opt/trn_rl_repo")
    from concourse.bass_utils import run_bass_kernel_spmd

    if "nc" not in _nc_cache:
        _nc_cache["nc"] = _build_nc()
    nc = _nc_cache["nc"]

    in_maps = make_core_inputs(**inputs)
    res = run_bass_kernel_spmd(nc, in_maps, core_ids=list(range(NCORES)),
                               trace=trace)
    out = np.zeros((T, C), np.float64)
    for r in res.results:
        out += r["opart"].astype(np.float64)
    return out.astype(np.float32).reshape(1, T, C), res


def kernel(**inputs):
    import sys
    if "/opt/trn_rl_repo" not in sys.path:
        sys.path.insert(0, "/opt/trn_rl_repo")
    out, _ = run(inputs)
    return out


# revision 3
# speedup vs baseline: 1.0086x; 1.0086x over previous
"""Causal self-attention with RoPE, tensor-parallel over heads on 8 trn2 cores.

Reference computation (B=1, T=4096, C=1024, h=16, d=64, fp32):
    q/k/v = x @ W{q,k,v}^T ; rope(q), rope(k) ; causal softmax(q k^T / 8) v ; @ Wo^T

Sharding: 2 heads per core (tensor parallel). Each core reads the full x
(transposed + bf16 on host) and its slice of Wq/Wk/Wv (column-parallel) and
Wo (row-parallel). Cores emit partial o-projections; the host sums them.

Device-side layout choices (v2):
  - qT/kT [dhead(=128 both heads) x T] with the head dim de-interleaved
    (rope real parts in partitions 0-31 / 64-95, imag in 32-63 / 96-127) so
    rope's pair-swap is a partition-block swap done by 4 small DMAs.
  - scores are computed transposed: sT[j, i] = sum_d kT[d,j] qT[d,i].
  - exp runs on ACT (hw Exp) or DVE (bf16 Schraudolph: bitcast(int16(
    x*A + B)), ~1% softmax-weight ripple that largely cancels between the
    softmax numerator and denominator); diagonal blocks fold the causal
    mask into a DVE scalar_tensor_tensor with a +B / -20000 tile (masked
    lanes land at ~1e-37 ~ 0), or run ACT exp + 0/1 mask multiply,
    restricted to the valid column range.  A static schedule balances the
    two engines.
  - att@v is computed with att as the *stationary* operand ([128k x 128q]
    blocks) and the ones-augmented v [128k x 65] moving, 65 PE cycles per
    block; the 4 query sub-blocks of a head accumulate into one psum bank
    as a single lazily-zeroed accumulation group (row 64 = softmax
    denominator); fully-masked diagonal sub-blocks are skipped.
  - normalize: per-query reciprocal of the denominators + scaled
    psum->sbuf copies per (qs, head); one batched XBAR dma-transpose per
    window rebuilds yT [dd x tok] for the o-projection (bf16), whose
    output rows go out in 256-row batched DMAs.
"""

import numpy as np
import ml_dtypes

bf16 = ml_dtypes.bfloat16

T, C, H, D = 4096, 1024, 16, 64
NCORES = 8
HPC = H // NCORES          # heads per core
DD = HPC * D               # per-core qkv features (=128)
P = 128

A0 = 128.0 / np.log(2.0)   # Schraudolph bf16: e^x ~ bitcast(int16(x*A0 + B0))
B0 = 127.0 * 128.0 - 7.0
SCALE = 0.125
MASKED = -20000.0

_nc_cache = {}


def _exp_schedule(nw):
    """(iw, jc) -> 'a' | 'd' (interior) | 'am' | 'dm' (diagonal)."""
    import os
    sched = {}
    k = dk = 0
    pat = tuple(os.environ.get("EXP_PAT", "adad"))
    dpat = tuple((os.environ.get("DIAG_PAT", "damd") + "m")[i] + "m"
                 for i in range(4)) if False else None
    dpat = tuple(c + "m" for c in os.environ.get("DIAG_PAT", "aada"))
    if os.environ.get("EXP_ALL") == "a":
        pat, dpat = ("a",), ("am",)
    elif os.environ.get("EXP_ALL") == "d":
        pat, dpat = ("d",), ("dm",)
    for iw in range(nw):
        njc = 4 * iw + 4
        for jc in range(njc):
            if jc >= 4 * iw:
                sched[(iw, jc)] = dpat[(jc - 4 * iw) % len(dpat)]
                dk += 1
            else:
                sched[(iw, jc)] = pat[k % len(pat)]
                k += 1
    return sched


def _build_nc(t=T):
    import concourse.bass as bass
    import concourse.tile as tile
    import concourse.mybir as mybir
    from concourse import bacc

    f32 = mybir.dt.float32
    b16 = mybir.dt.bfloat16
    i16 = mybir.dt.int16
    MUL = mybir.AluOpType.mult
    ADD = mybir.AluOpType.add
    EXP = mybir.ActivationFunctionType.Exp
    CPY = mybir.ActivationFunctionType.Copy

    nt = t // 512            # qkv t-chunks
    nw = t // 512            # attention query windows
    njb = t // P             # key blocks
    AS = SCALE * A0

    sched = _exp_schedule(nw)

    nc = bacc.Bacc("TRN2")

    xt_d = nc.dram_tensor("xt", [C, t], b16, kind="ExternalInput")
    # weights host-prepacked to [P, C//P, DD]: one contiguous 2KB run per
    # partition (full-rate DMA descriptors)
    wq_d = nc.dram_tensor("wq", [P, C // P, DD], b16, kind="ExternalInput")
    wk_d = nc.dram_tensor("wk", [P, C // P, DD], b16, kind="ExternalInput")
    wv_d = nc.dram_tensor("wv", [P, C // P, DD], b16, kind="ExternalInput")
    wo_d = nc.dram_tensor("wo", [DD, C], b16, kind="ExternalInput")
    cos_d = nc.dram_tensor("cosb", [P, t], b16, kind="ExternalInput")
    sin_d = nc.dram_tensor("sinb", [P, t], b16, kind="ExternalInput")
    msk_d = nc.dram_tensor("mask4", [P, 4, 512], b16, kind="ExternalInput")
    mskb_d = nc.dram_tensor("maskb", [P, 4, 512], f32, kind="ExternalInput")
    out_d = nc.dram_tensor("opart", [t, C], b16, kind="ExternalOutput")

    with tile.TileContext(nc) as tc:
        with (
            tc.tile_pool(name="const", bufs=1) as constp,
            tc.tile_pool(name="xload", bufs=3) as xload,
            tc.tile_pool(name="rope", bufs=3) as ropep,
            tc.tile_pool(name="att", bufs=4) as attp,
            tc.tile_pool(name="small", bufs=4) as smallp,
            tc.tile_pool(name="ps", bufs=1, space="PSUM") as psp,
        ):
            # ---- constants / persistent tensors. DMA issue order chosen so
            # the first qkv matmuls (wq + x chunk 0) start ASAP.
            xt_pre = {}

            def load_xt(tch):
                tsl = slice(tch * 512, (tch + 1) * 512)
                xt = xload.tile([P, C // P, 512], b16, name="xt", tag="xt")
                nc.sync.dma_start(
                    xt, xt_d[:].rearrange("(co p) t -> p co t", p=P)[:, :, tsl]
                )
                xt_pre[tch] = xt

            wq_sb = constp.tile([P, C // P, DD], b16)
            nc.sync.dma_start(wq_sb, wq_d[:])
            # x chunk 0 in four slices so the first qkv matmuls start sooner
            xt0 = xload.tile([P, C // P, 512], b16, name="xt0")
            xt_view = xt_d[:].rearrange("(co p) t -> p co t", p=P)
            for c4 in range(4):
                nc.sync.dma_start(xt0[:, 2 * c4:2 * c4 + 2],
                                  xt_view[:, 2 * c4:2 * c4 + 2, 0:512])
            xt_pre[0] = xt0
            wk_sb = constp.tile([P, C // P, DD], b16)
            nc.sync.dma_start(wk_sb, wk_d[:])
            load_xt(1)
            wv_sb = constp.tile([P, C // P, DD], b16)
            nc.sync.dma_start(wv_sb, wv_d[:])
            # rope tables: first two chunks' columns now, tails late
            cos_sb = constp.tile([P, t], b16)
            nc.sync.dma_start(cos_sb[:, 0:1024], cos_d[:, 0:1024])
            sin_sb = constp.tile([P, t], b16)
            nc.sync.dma_start(sin_sb[:, 0:1024], sin_d[:, 0:1024])
            # the first window's diagonal needs maskb[0] immediately
            mskb_sb = constp.tile([P, 4, 512], f32)
            nc.sync.dma_start(mskb_sb[:, 0:1], mskb_d[:, 0:1])
            msk_sb = constp.tile([P, 4, 512], b16)
            nc.sync.dma_start(msk_sb[:, 0:1], msk_d[:, 0:1])
            wo_sb = constp.tile([DD, C], b16)

            def late_consts():
                nc.sync.dma_start(mskb_sb[:, 1:4], mskb_d[:, 1:4])
                nc.sync.dma_start(msk_sb[:, 1:4], msk_d[:, 1:4])
                if t > 1024:
                    nc.sync.dma_start(cos_sb[:, 1024:], cos_d[:, 1024:])
                    nc.sync.dma_start(sin_sb[:, 1024:], sin_d[:, 1024:])
                nc.sync.dma_start(wo_sb, wo_d[:])

            qT = constp.tile([P, t], b16)   # rope'd q, both heads
            kT = constp.tile([P, t], b16)
            # v in natural layout per 128-block, +ones cols at 64 and 129
            vaug = constp.tile([P, njb, 2 * D + 2], b16)
            nc.vector.memset(vaug[:, :, D], 1.0)
            nc.vector.memset(vaug[:, :, 2 * D + 1], 1.0)

            # PSUM budget (8 banks): pss2 4KB x3 (scores, triple-buffered;
            # qkv and o-proj psums borrow half-slots from the same rotation)
            # + psy0/psy1 2KB x1 (att@v + denominator accumulators).
            def scratch():
                return psp.tile([P, 512], f32, tag="scratch", bufs=2,
                                name="scr")

            ob_k = {"k": 0}

            def drain_copy(dst, src_):
                """psum->sbuf copy alternating ACT/DVE."""
                ob_k["k"] += 1
                import os as _o4
                pat4 = _o4.environ.get("DRAIN", "ad")
                if pat4[ob_k["k"] % len(pat4)] == "a":
                    nc.scalar.copy(dst, src_)
                else:
                    nc.vector.tensor_copy(dst, src_)

            def qkv_parts(tch, cp):
                """Six micro-closures (~0.85us of PE each) spread through an
                attention window's PE slack. cp = engine for the psum->sbuf
                copies (ACT before the windows saturate it, DVE after)."""
                tsl = slice(tch * 512, (tch + 1) * 512)
                st = {}
                import os as _o3
                reng = nc.gpsimd if (
                    _o3.environ.get("GPSROPE", "1") == "1" and tch >= int(_o3.environ.get("GPSROPE_MIN", "0"))) \
                    else nc.vector

                def copy(dst, src_):
                    if cp is nc.scalar:
                        nc.scalar.copy(dst, src_)
                    else:
                        cp.tensor_copy(dst, src_)

                def mm_half(name, w_sb, half):
                    if name == "q" and half == 0:
                        st["xt"] = xt_pre.pop(tch)
                        st["q"] = scratch()
                    if name == "k" and half == 0:
                        if tch + 2 < nt and tch + 2 not in xt_pre:
                            load_xt(tch + 2)
                    if half == 0 and name != "q":
                        st[name] = scratch()
                    ps = st[name]
                    for ci in range(4 * half, 4 * half + 4):
                        nc.tensor.matmul(
                            ps, w_sb[:, ci], st["xt"][:, ci],
                            start=(ci == 0), stop=(ci == C // P - 1),
                        )

                def q1():
                    mm_half("q", wq_sb, 0)

                def q2():
                    mm_half("q", wq_sb, 1)
                    # q and k share one [128, 2, 512] tile; the rope
                    # pair-swap is partition-block DMAs issued right after
                    # each tensor's psum copy so the swap latency overlaps
                    # the other tensor's matmuls
                    st["qf2"] = ropep.tile([P, 2, 512], b16, tag="qf2",
                                           name="qf2")
                    st["sw2"] = ropep.tile([P, 2, 512], b16, tag="sw2",
                                           name="sw2")
                    copy(st["qf2"][:, 0], st["q"])
                    import os as _o2
                    if _o2.environ.get("EARLYSWAP") == "1":
                        qf2, sw2 = st["qf2"], st["sw2"]
                        nc.sync.dma_start(sw2[0:32, 0], qf2[32:64, 0])
                        nc.sync.dma_start(sw2[32:64, 0], qf2[0:32, 0])
                        nc.sync.dma_start(sw2[64:96, 0], qf2[96:128, 0])
                        nc.sync.dma_start(sw2[96:128, 0], qf2[64:96, 0])

                def k1():
                    mm_half("k", wk_sb, 0)

                def k2():
                    mm_half("k", wk_sb, 1)
                    qf2 = st["qf2"]
                    sw2 = st["sw2"]
                    copy(qf2[:, 1], st["k"])
                    import os as _o2
                    if _o2.environ.get("EARLYSWAP") == "1":
                        nc.sync.dma_start(sw2[0:32, 1], qf2[32:64, 1])
                        nc.sync.dma_start(sw2[32:64, 1], qf2[0:32, 1])
                        nc.sync.dma_start(sw2[64:96, 1], qf2[96:128, 1])
                        nc.sync.dma_start(sw2[96:128, 1], qf2[64:96, 1])
                    else:
                        nc.sync.dma_start(sw2[0:32], qf2[32:64])
                        nc.sync.dma_start(sw2[32:64], qf2[0:32])
                        nc.sync.dma_start(sw2[64:96], qf2[96:128])
                        nc.sync.dma_start(sw2[96:128], qf2[64:96])
                    for i, name in enumerate(("q", "k")):
                        tl = ropep.tile([P, 512], b16, tag=f"t1_{name}",
                                        name="t1")
                        reng.tensor_tensor(tl, qf2[:, i],
                                           cos_sb[:, tsl], MUL)
                        st[f"t1{name}"] = tl

                def v1():
                    t2q = ropep.tile([P, 512], b16, tag="t2_q", name="t2")
                    reng.tensor_tensor(t2q, st["sw2"][:, 0],
                                       sin_sb[:, tsl], MUL)
                    reng.tensor_add(qT[:, tsl], st["t1q"], t2q)
                    mm_half("v", wv_sb, 0)

                def v2():
                    mm_half("v", wv_sb, 1)
                    t2k = ropep.tile([P, 512], b16, tag="t2_k", name="t2")
                    reng.tensor_tensor(t2k, st["sw2"][:, 1],
                                       sin_sb[:, tsl], MUL)
                    reng.tensor_add(kT[:, tsl], st["t1k"], t2k)
                    # v: psum->sbuf copy, XBAR dma-transpose to dense [t, d]
                    # blocks, two narrow gpsimd copies around the ones column
                    vt = ropep.tile([P, 512], b16, tag="vt", name="vt")
                    copy(vt, st["v"])
                    dense = ropep.tile([P, 4, P], b16, tag="vdense",
                                       name="vdense")
                    nc.sync.dma_start_transpose(dense, vt[:])
                    g0 = tch * 4
                    nc.gpsimd.tensor_copy(vaug[:, g0:g0 + 4, 0:D],
                                          dense[:, :, 0:D])
                    nc.gpsimd.tensor_copy(vaug[:, g0:g0 + 4, D + 1:2 * D + 1],
                                          dense[:, :, D:2 * D])

                return [q1, q2, k1, k2, v1, v2]

            def qkv_chunk(tch):
                for f in qkv_parts(tch, nc.scalar):
                    f()

            def oproj_fillers(iw, yT, tail=False):
                """o-projection for two 256-row blocks, one batched output
                DMA each. Stationary yT blocks come from the window's XBAR
                transpose."""
                outs = []
                for pi in range(2):
                    def one(pi=pi):
                        ob4 = attp.tile([P, 2, 2, 512], b16, tag="ob4",
                                        bufs=2, name="ob4")
                        for j in range(2):
                            g = 2 * pi + j
                            for mc in range(2):
                                pso = scratch()
                                nc.tensor.matmul(
                                    pso, yT[:, g],
                                    wo_sb[:, mc * 512:(mc + 1) * 512],
                                    start=True, stop=True,
                                )
                                drain_copy(ob4[:, j, mc], pso)
                        tb0 = 4 * iw + 2 * pi
                        dst = out_d[tb0 * P:(tb0 + 2) * P, :].rearrange(
                            "(j p) (mc i) -> p j mc i", j=2, mc=2)
                        nc.sync.dma_start(dst, ob4)
                    outs.append(one)
                return outs

            def win512(iw, parts=(), oproj=(), last=False, head=()):
                """512-wide attention window over i in [512iw, 512iw+512),
                one-jc software-pipelined. Fillers run in the slack of the
                jc loop: qkv chunk micro-parts from jc>=1, the o-projection
                of window iw-2 from jc>=3."""
                isl = slice(iw * 512, (iw + 1) * 512)
                njc = 4 * iw + 4
                parts = list(parts)
                oproj = list(oproj)
                psy = [psp.tile([P, 4, P], f32, tag=f"psy{h}",
                                name="psy") for h in range(HPC)]

                def score_exp(jc, isl_, oiw):
                    eng = sched[(oiw, jc)]
                    m = jc - 4 * oiw if jc >= 4 * oiw else None
                    import os as _o
                    lo = m * P if (m is not None and _o.environ.get(
                        "DIAG_SCORE") == "1") else 0
                    jsl = slice(jc * P, (jc + 1) * P)
                    ps2 = psp.tile([P, 2, 512], f32, tag="pss2", bufs=2,
                                   name="ps2")
                    for h in range(HPC):
                        hb = D * h
                        nc.tensor.matmul(
                            ps2[:, h, lo:],
                            kT[hb:hb + D, jsl],
                            qT[hb:hb + D, isl_.start + lo:isl_.stop],
                            start=True, stop=True)
                    att2 = attp.tile([P, 2, 512], b16, tag="att2",
                                     name="att2", bufs=10)
                    if m is None:
                        if _o.environ.get("EXPSPLIT") == "1":
                            nc.scalar.activation(att2[:, 0], ps2[:, 0], EXP,
                                                 scale=SCALE)
                            nc.vector.tensor_scalar(
                                att2[:, 1].bitcast(i16), ps2[:, 1],
                                AS, B0, MUL, ADD)
                        elif eng == "a":
                            nc.scalar.activation(att2, ps2, EXP, scale=SCALE)
                        else:
                            nc.vector.tensor_scalar(att2[:].bitcast(i16),
                                                    ps2, AS, B0, MUL, ADD)
                    else:
                        lo = m * P
                        if eng == "am":
                            nc.scalar.activation(att2[:, :, lo:],
                                                 ps2[:, :, lo:], EXP,
                                                 scale=SCALE)
                            for h in range(HPC):
                                nc.vector.tensor_tensor(
                                    att2[:, h, lo:], att2[:, h, lo:],
                                    msk_sb[:, m, lo:], MUL)
                        else:
                            for h in range(HPC):
                                nc.vector.scalar_tensor_tensor(
                                    att2[:, h, lo:].bitcast(i16),
                                    ps2[:, h, lo:], AS,
                                    mskb_sb[:, m, lo:], MUL, ADD)
                    return att2, m

                def emit_attv(p):
                    jc0, a2, m0 = p
                    for h in range(HPC):
                        va = vaug[:, jc0, 65 * h:65 * h + 65]
                        for qs in range(4):
                            if m0 is not None and qs < m0:
                                continue
                            nc.tensor.matmul(
                                psy[h][:, qs, 0:65],
                                a2[:, h, qs * P:(qs + 1) * P], va,
                                start=(jc0 == 0 and qs == 0),
                                stop=(jc0 == njc - 1 and qs == 3))

                pend = list(head)
                for jc in range(len(head), njc):
                    att2, m = score_exp(jc, isl, iw)
                    if len(pend) == int(__import__("os").environ.get(
                            "PEND", "2")):
                        emit_attv(pend.pop(0))
                    pend.append((jc, att2, m))
                    if jc < njc - 3:
                        if jc >= 1 and parts:
                            parts.pop(0)()
                        elif jc >= 3 and oproj and (
                                jc % 2 == 1
                                or 3 * len(oproj) > njc - 3 - jc):
                            oproj.pop(0)()
                nxt = []
                if not last and iw > 0:
                    # pre-compute the next window's first score/exp blocks
                    # so PE has work while this window's tail drains
                    isl_n = slice((iw + 1) * 512, (iw + 2) * 512)
                    import os as _o6
                    for jc_n in range(int(_o6.environ.get("NXT", "3"))):
                        a2n, mn = score_exp(jc_n, isl_n, iw + 1)
                        nxt.append((jc_n, a2n, mn))
                for p in pend:
                    emit_attv(p)
                recs = []
                for h in range(HPC):
                    rec = smallp.tile([P, 4], f32, tag=f"rec{h}", bufs=3,
                                      name="rec")
                    nc.vector.reciprocal(rec, psy[h][:, :, 64])
                    recs.append(rec)
                yT = ropep.tile([P, 4, P], b16, tag="yt", bufs=3, name="yT")

                def norm_half(q0):
                    def f():
                        ysb = attp.tile([P, 2, P], b16, tag=f"ysb{q0}",
                                        bufs=2, name="ysb")
                        for qs in (q0, q0 + 1):
                            for h in range(HPC):
                                dst = ysb[:, qs - q0, 64 * h:64 * h + 64]
                                srcp = psy[h][:, qs, 0:64]
                                import os as _o7
                                nrm = _o7.environ.get("NORMENG", "a")
                                if nrm == "a" or (nrm == "alt"
                                                  and (qs + h) % 2 == 0):
                                    nc.scalar.activation(
                                        dst, srcp, CPY,
                                        scale=recs[h][:, qs:qs + 1])
                                else:
                                    nc.vector.tensor_scalar(
                                        dst, srcp, recs[h][:, qs:qs + 1],
                                        None, MUL)
                        nc.sync.dma_start_transpose(
                            yT[:, q0:q0 + 2, :],
                            ysb[:].rearrange("p q d -> p (q d)"))
                    return f

                import os
                if os.environ.get("OLD_NORM") == "1":
                    ysb = attp.tile([P, 4, P], b16, tag="ysb0", bufs=2,
                                    name="ysb")
                    for qs in range(4):
                        for h in range(HPC):
                            dst = ysb[:, qs, 64 * h:64 * h + 64]
                            srcp = psy[h][:, qs, 0:64]
                            if (qs + h) % 2 == 0:
                                nc.scalar.activation(
                                    dst, srcp, CPY,
                                    scale=recs[h][:, qs:qs + 1])
                            else:
                                nc.vector.tensor_scalar(
                                    dst, srcp, recs[h][:, qs:qs + 1],
                                    None, MUL)
                    nc.sync.dma_start_transpose(
                        yT, ysb[:].rearrange("p q d -> p (q d)"))
                    norms = []
                else:
                    norms = [norm_half(0), norm_half(2)]
                if norms and (last or os.environ.get("NO_DEFER_NORM") == "1"):
                    for f in norms:
                        f()
                    norms = []
                for f in parts + oproj:   # leftovers
                    f()
                return oproj_fillers(iw, yT, tail=last), nxt, norms

            qkv_chunk(0)
            c1 = qkv_parts(1, nc.scalar)
            for f in c1[:4]:
                f()
            late_consts()

            pend_ops = {}
            hd = ()
            norms = []
            for iw in range(nw):
                if iw == 0:
                    parts = c1[4:] + (qkv_parts(2, nc.scalar)
                                      if nt > 2 else [])
                else:
                    tch = iw + 2
                    import os as _o5
                    _cpm = int(_o5.environ.get("CPMAX", "7"))
                    parts = qkv_parts(
                        tch, nc.scalar if tch <= _cpm else nc.vector) \
                        if tch < nt else []
                if iw == nw - 2:
                    oproj = pend_ops.pop(iw - 2, []) + pend_ops.pop(iw - 1,
                                                                    [])
                elif iw == nw - 1:
                    oproj = pend_ops.pop(iw - 1, [])
                else:
                    oproj = pend_ops.pop(iw - 2, [])
                ops, hd, norms = win512(iw, parts=norms + parts,
                                        oproj=oproj,
                                        last=(iw == nw - 1), head=hd)
                pend_ops[iw] = ops
            for f in pend_ops.pop(nw - 1):
                f()

    nc.compile()
    return nc


def _perm_deinterleave():
    """Row permutation for Wq/Wk: per head, even rows then odd rows."""
    perm = []
    for h in range(H):
        base = h * D
        perm += [base + 2 * k for k in range(D // 2)]
        perm += [base + 2 * k + 1 for k in range(D // 2)]
    return np.array(perm)


def make_core_inputs(x, freqs_cos, freqs_sin, Wq, Wk, Wv, Wo, t=T):
    """Host-side sharding/layout prep. Returns per-core input dicts."""
    x = np.asarray(x, np.float32).reshape(t, C)
    fc = np.asarray(freqs_cos, np.float32)
    fs = np.asarray(freqs_sin, np.float32)
    Wq = np.asarray(Wq, np.float32)
    Wk = np.asarray(Wk, np.float32)
    Wv = np.asarray(Wv, np.float32)
    Wo = np.asarray(Wo, np.float32)

    xt = np.ascontiguousarray(x.T).astype(bf16)                  # [C, t]
    perm = _perm_deinterleave()
    Wq_p, Wk_p = Wq[perm], Wk[perm]

    # rope factor tables in the de-interleaved [dd, t] layout
    kidx = np.arange(P) % 32
    sgn = np.where((np.arange(P) // 32) % 2 == 0, -1.0, 1.0).astype(np.float32)
    cosb = fc.T[kidx].astype(bf16)                               # [128, t]
    sinb = (fs.T[kidx] * sgn[:, None]).astype(bf16)

    # diagonal-tile causal masks: valid iff 128*m + j <= i
    jj = np.arange(P)[:, None, None]
    mm = np.arange(4)[None, :, None]
    ii = np.arange(512)[None, None, :]
    valid = (P * mm + jj) <= ii
    mask4 = valid.astype(bf16)
    maskb = np.where(valid, np.float32(B0), np.float32(MASKED))

    def pack_w(w):
        # [C, DD] -> [P, C//P, DD]: one contiguous 2KB run per partition
        return np.ascontiguousarray(
            w.T.reshape(C // P, P, DD).transpose(1, 0, 2)).astype(bf16)

    in_maps = []
    for c in range(NCORES):
        rows = slice(c * DD, (c + 1) * DD)
        in_maps.append({
            "xt": xt,
            "wq": pack_w(Wq_p[rows]),
            "wk": pack_w(Wk_p[rows]),
            "wv": pack_w(Wv[rows]),
            "wo": np.ascontiguousarray(Wo[:, rows].T).astype(bf16),
            "cosb": cosb,
            "sinb": sinb,
            "mask4": mask4,
            "maskb": maskb,
        })
    return in_maps


def run(inputs, trace=False):
    """Compile once, run on 8 cores, host-sum partials."""
    import sys
    if "/opt/trn_rl_repo" not in sys.path:
        sys.path.insert(0, "/opt/trn_rl_repo")
    from concourse.bass_utils import run_bass_kernel_spmd

    if "nc" not in _nc_cache:
        _nc_cache["nc"] = _build_nc()
    nc = _nc_cache["nc"]

    in_maps = make_core_inputs(**inputs)
    res = run_bass_kernel_spmd(nc, in_maps, core_ids=list(range(NCORES)),
                               trace=trace)
    out = np.zeros((T, C), np.float64)
    for r in res.results:
        out += r["opart"].astype(np.float64)
    return out.astype(np.float32).reshape(1, T, C), res


def kernel(**inputs):
    import sys
    if "/opt/trn_rl_repo" not in sys.path:
        sys.path.insert(0, "/opt/trn_rl_repo")
    out, _ = run(inputs)
    return out


# revision 4
# speedup vs baseline: 1.0131x; 1.0045x over previous
"""Causal self-attention with RoPE, tensor-parallel over heads on 8 trn2 cores.

Reference computation (B=1, T=4096, C=1024, h=16, d=64, fp32):
    q/k/v = x @ W{q,k,v}^T ; rope(q), rope(k) ; causal softmax(q k^T / 8) v ; @ Wo^T

Sharding: 2 heads per core (tensor parallel). Each core reads the full x
(transposed + bf16 on host) and its slice of Wq/Wk/Wv (column-parallel) and
Wo (row-parallel). Cores emit partial o-projections; the host sums them.

Device-side layout choices (v2):
  - qT/kT [dhead(=128 both heads) x T] with the head dim de-interleaved
    (rope real parts in partitions 0-31 / 64-95, imag in 32-63 / 96-127) so
    rope's pair-swap is a partition-block swap done by 4 small DMAs.
  - scores are computed transposed: sT[j, i] = sum_d kT[d,j] qT[d,i].
  - exp runs on ACT (hw Exp) or DVE (bf16 Schraudolph: bitcast(int16(
    x*A + B)), ~1% softmax-weight ripple that largely cancels between the
    softmax numerator and denominator); diagonal blocks fold the causal
    mask into a DVE scalar_tensor_tensor with a +B / -20000 tile (masked
    lanes land at ~1e-37 ~ 0), or run ACT exp + 0/1 mask multiply,
    restricted to the valid column range.  A static schedule balances the
    two engines.
  - att@v is computed with att as the *stationary* operand ([128k x 128q]
    blocks) and the ones-augmented v [128k x 65] moving, 65 PE cycles per
    block; the 4 query sub-blocks of a head accumulate into one psum bank
    as a single lazily-zeroed accumulation group (row 64 = softmax
    denominator); fully-masked diagonal sub-blocks are skipped.
  - normalize: per-query reciprocal of the denominators + scaled
    psum->sbuf copies per (qs, head); one batched XBAR dma-transpose per
    window rebuilds yT [dd x tok] for the o-projection (bf16), whose
    output rows go out in 256-row batched DMAs.
"""

import numpy as np
import ml_dtypes

bf16 = ml_dtypes.bfloat16

T, C, H, D = 4096, 1024, 16, 64
NCORES = 8
HPC = H // NCORES          # heads per core
DD = HPC * D               # per-core qkv features (=128)
P = 128

A0 = 128.0 / np.log(2.0)   # Schraudolph bf16: e^x ~ bitcast(int16(x*A0 + B0))
B0 = 127.0 * 128.0 - 7.0
SCALE = 0.125
MASKED = -20000.0

_nc_cache = {}


def _exp_schedule(nw):
    """(iw, jc) -> 'a' | 'd' (interior) | 'am' | 'dm' (diagonal)."""
    import os
    sched = {}
    k = dk = 0
    pat = tuple(os.environ.get("EXP_PAT", "adad"))
    dpat = tuple((os.environ.get("DIAG_PAT", "damd") + "m")[i] + "m"
                 for i in range(4)) if False else None
    dpat = tuple(c + "m" for c in os.environ.get("DIAG_PAT", "aada"))
    if os.environ.get("EXP_ALL") == "a":
        pat, dpat = ("a",), ("am",)
    elif os.environ.get("EXP_ALL") == "d":
        pat, dpat = ("d",), ("dm",)
    for iw in range(nw):
        njc = 4 * iw + 4
        for jc in range(njc):
            if jc >= 4 * iw:
                sched[(iw, jc)] = dpat[(jc - 4 * iw) % len(dpat)]
                dk += 1
            else:
                sched[(iw, jc)] = pat[k % len(pat)]
                k += 1
    return sched


def _build_nc(t=T):
    import concourse.bass as bass
    import concourse.tile as tile
    import concourse.mybir as mybir
    from concourse import bacc

    f32 = mybir.dt.float32
    b16 = mybir.dt.bfloat16
    i16 = mybir.dt.int16
    MUL = mybir.AluOpType.mult
    ADD = mybir.AluOpType.add
    EXP = mybir.ActivationFunctionType.Exp
    CPY = mybir.ActivationFunctionType.Copy

    nt = t // 512            # qkv t-chunks
    nw = t // 512            # attention query windows
    njb = t // P             # key blocks
    AS = SCALE * A0

    sched = _exp_schedule(nw)

    nc = bacc.Bacc("TRN2")

    xt_d = nc.dram_tensor("xt", [C, t], b16, kind="ExternalInput")
    # weights host-prepacked to [P, C//P, DD]: one contiguous 2KB run per
    # partition (full-rate DMA descriptors)
    wq_d = nc.dram_tensor("wq", [P, C // P, DD], b16, kind="ExternalInput")
    wk_d = nc.dram_tensor("wk", [P, C // P, DD], b16, kind="ExternalInput")
    wv_d = nc.dram_tensor("wv", [P, C // P, DD], b16, kind="ExternalInput")
    wo_d = nc.dram_tensor("wo", [DD, C], b16, kind="ExternalInput")
    cos_d = nc.dram_tensor("cosb", [P, t], b16, kind="ExternalInput")
    sin_d = nc.dram_tensor("sinb", [P, t], b16, kind="ExternalInput")
    msk_d = nc.dram_tensor("mask4", [P, 4, 512], b16, kind="ExternalInput")
    mskb_d = nc.dram_tensor("maskb", [P, 4, 512], f32, kind="ExternalInput")
    out_d = nc.dram_tensor("opart", [t, C], b16, kind="ExternalOutput")

    with tile.TileContext(nc) as tc:
        with (
            tc.tile_pool(name="const", bufs=1) as constp,
            tc.tile_pool(name="xload", bufs=3) as xload,
            tc.tile_pool(name="rope", bufs=3) as ropep,
            tc.tile_pool(name="att", bufs=4) as attp,
            tc.tile_pool(name="small", bufs=4) as smallp,
            tc.tile_pool(name="ps", bufs=1, space="PSUM") as psp,
        ):
            # ---- constants / persistent tensors. DMA issue order chosen so
            # the first qkv matmuls (wq + x chunk 0) start ASAP.
            xt_pre = {}

            def load_xt(tch):
                tsl = slice(tch * 512, (tch + 1) * 512)
                xt = xload.tile([P, C // P, 512], b16, name="xt", tag="xt")
                nc.sync.dma_start(
                    xt, xt_d[:].rearrange("(co p) t -> p co t", p=P)[:, :, tsl]
                )
                xt_pre[tch] = xt

            wq_sb = constp.tile([P, C // P, DD], b16)
            nc.sync.dma_start(wq_sb, wq_d[:])
            # x chunk 0 in four slices so the first qkv matmuls start sooner
            xt0 = xload.tile([P, C // P, 512], b16, name="xt0")
            xt_view = xt_d[:].rearrange("(co p) t -> p co t", p=P)
            for c4 in range(4):
                nc.sync.dma_start(xt0[:, 2 * c4:2 * c4 + 2],
                                  xt_view[:, 2 * c4:2 * c4 + 2, 0:512])
            xt_pre[0] = xt0
            wk_sb = constp.tile([P, C // P, DD], b16)
            nc.sync.dma_start(wk_sb, wk_d[:])
            load_xt(1)
            wv_sb = constp.tile([P, C // P, DD], b16)
            nc.sync.dma_start(wv_sb, wv_d[:])
            # rope tables: first two chunks' columns now, tails late
            cos_sb = constp.tile([P, t], b16)
            nc.sync.dma_start(cos_sb[:, 0:1024], cos_d[:, 0:1024])
            sin_sb = constp.tile([P, t], b16)
            nc.sync.dma_start(sin_sb[:, 0:1024], sin_d[:, 0:1024])
            # the first window's diagonal needs maskb[0] immediately
            mskb_sb = constp.tile([P, 4, 512], f32)
            nc.sync.dma_start(mskb_sb[:, 0:1], mskb_d[:, 0:1])
            msk_sb = constp.tile([P, 4, 512], b16)
            nc.sync.dma_start(msk_sb[:, 0:1], msk_d[:, 0:1])
            wo_sb = constp.tile([DD, C], b16)

            def late_consts():
                nc.sync.dma_start(mskb_sb[:, 1:4], mskb_d[:, 1:4])
                nc.sync.dma_start(msk_sb[:, 1:4], msk_d[:, 1:4])
                if t > 1024:
                    nc.sync.dma_start(cos_sb[:, 1024:], cos_d[:, 1024:])
                    nc.sync.dma_start(sin_sb[:, 1024:], sin_d[:, 1024:])
                nc.sync.dma_start(wo_sb, wo_d[:])

            qT = constp.tile([P, t], b16)   # rope'd q, both heads
            kT = constp.tile([P, t], b16)
            # v in natural layout per 128-block, +ones cols at 64 and 129
            vaug = constp.tile([P, njb, 2 * D + 2], b16)
            nc.vector.memset(vaug[:, :, D], 1.0)
            nc.vector.memset(vaug[:, :, 2 * D + 1], 1.0)

            # PSUM budget (8 banks): pss2 4KB x3 (scores, triple-buffered;
            # qkv and o-proj psums borrow half-slots from the same rotation)
            # + psy0/psy1 2KB x1 (att@v + denominator accumulators).
            def scratch():
                return psp.tile([P, 512], f32, tag="scratch", bufs=2,
                                name="scr")

            ob_k = {"k": 0}

            def drain_copy(dst, src_):
                """psum->sbuf copy alternating ACT/DVE."""
                ob_k["k"] += 1
                import os as _o4
                pat4 = _o4.environ.get("DRAIN", "ad")
                if pat4[ob_k["k"] % len(pat4)] == "a":
                    nc.scalar.copy(dst, src_)
                else:
                    nc.vector.tensor_copy(dst, src_)

            def qkv_parts(tch, cp):
                """Six micro-closures (~0.85us of PE each) spread through an
                attention window's PE slack. cp = engine for the psum->sbuf
                copies (ACT before the windows saturate it, DVE after)."""
                tsl = slice(tch * 512, (tch + 1) * 512)
                st = {}
                import os as _o3
                reng = nc.gpsimd if (
                    _o3.environ.get("GPSROPE", "1") == "1" and tch >= int(_o3.environ.get("GPSROPE_MIN", "0"))) \
                    else nc.vector

                def copy(dst, src_):
                    if cp is nc.scalar:
                        nc.scalar.copy(dst, src_)
                    else:
                        cp.tensor_copy(dst, src_)

                def mm_half(name, w_sb, half):
                    if name == "q" and half == 0:
                        st["xt"] = xt_pre.pop(tch)
                        st["q"] = scratch()
                    if name == "k" and half == 0:
                        if tch + 2 < nt and tch + 2 not in xt_pre:
                            load_xt(tch + 2)
                    if half == 0 and name != "q":
                        st[name] = scratch()
                    ps = st[name]
                    for ci in range(4 * half, 4 * half + 4):
                        nc.tensor.matmul(
                            ps, w_sb[:, ci], st["xt"][:, ci],
                            start=(ci == 0), stop=(ci == C // P - 1),
                        )

                def q1():
                    mm_half("q", wq_sb, 0)

                def q2():
                    mm_half("q", wq_sb, 1)
                    # q and k share one [128, 2, 512] tile; the rope
                    # pair-swap is partition-block DMAs issued right after
                    # each tensor's psum copy so the swap latency overlaps
                    # the other tensor's matmuls
                    st["qf2"] = ropep.tile([P, 2, 512], b16, tag="qf2",
                                           name="qf2")
                    st["sw2"] = ropep.tile([P, 2, 512], b16, tag="sw2",
                                           name="sw2")
                    copy(st["qf2"][:, 0], st["q"])
                    import os as _o2
                    if _o2.environ.get("EARLYSWAP") == "1":
                        qf2, sw2 = st["qf2"], st["sw2"]
                        nc.sync.dma_start(sw2[0:32, 0], qf2[32:64, 0])
                        nc.sync.dma_start(sw2[32:64, 0], qf2[0:32, 0])
                        nc.sync.dma_start(sw2[64:96, 0], qf2[96:128, 0])
                        nc.sync.dma_start(sw2[96:128, 0], qf2[64:96, 0])

                def k1():
                    mm_half("k", wk_sb, 0)

                def k2():
                    mm_half("k", wk_sb, 1)
                    qf2 = st["qf2"]
                    sw2 = st["sw2"]
                    copy(qf2[:, 1], st["k"])
                    import os as _o2
                    if _o2.environ.get("EARLYSWAP") == "1":
                        nc.sync.dma_start(sw2[0:32, 1], qf2[32:64, 1])
                        nc.sync.dma_start(sw2[32:64, 1], qf2[0:32, 1])
                        nc.sync.dma_start(sw2[64:96, 1], qf2[96:128, 1])
                        nc.sync.dma_start(sw2[96:128, 1], qf2[64:96, 1])
                    else:
                        nc.sync.dma_start(sw2[0:32], qf2[32:64])
                        nc.sync.dma_start(sw2[32:64], qf2[0:32])
                        nc.sync.dma_start(sw2[64:96], qf2[96:128])
                        nc.sync.dma_start(sw2[96:128], qf2[64:96])
                    for i, name in enumerate(("q", "k")):
                        tl = ropep.tile([P, 512], b16, tag=f"t1_{name}",
                                        name="t1")
                        reng.tensor_tensor(tl, qf2[:, i],
                                           cos_sb[:, tsl], MUL)
                        st[f"t1{name}"] = tl

                def v1():
                    t2q = ropep.tile([P, 512], b16, tag="t2_q", name="t2")
                    reng.tensor_tensor(t2q, st["sw2"][:, 0],
                                       sin_sb[:, tsl], MUL)
                    reng.tensor_add(qT[:, tsl], st["t1q"], t2q)
                    mm_half("v", wv_sb, 0)

                def v2():
                    mm_half("v", wv_sb, 1)
                    t2k = ropep.tile([P, 512], b16, tag="t2_k", name="t2")
                    reng.tensor_tensor(t2k, st["sw2"][:, 1],
                                       sin_sb[:, tsl], MUL)
                    reng.tensor_add(kT[:, tsl], st["t1k"], t2k)
                    # v: psum->sbuf copy, XBAR dma-transpose to dense [t, d]
                    # blocks, two narrow gpsimd copies around the ones column
                    vt = ropep.tile([P, 512], b16, tag="vt", name="vt")
                    copy(vt, st["v"])
                    dense = ropep.tile([P, 4, P], b16, tag="vdense",
                                       name="vdense")
                    nc.sync.dma_start_transpose(dense, vt[:])
                    g0 = tch * 4
                    nc.gpsimd.tensor_copy(vaug[:, g0:g0 + 4, 0:D],
                                          dense[:, :, 0:D])
                    nc.gpsimd.tensor_copy(vaug[:, g0:g0 + 4, D + 1:2 * D + 1],
                                          dense[:, :, D:2 * D])

                return [q1, q2, k1, k2, v1, v2]

            def qkv_chunk(tch):
                for f in qkv_parts(tch, nc.scalar):
                    f()

            def oproj_fillers(iw, yT, tail=False):
                """o-projection for two 256-row blocks, one batched output
                DMA each. Stationary yT blocks come from the window's XBAR
                transpose."""
                outs = []
                for pi in range(2):
                    def one(pi=pi):
                        ob4 = attp.tile([P, 2, 2, 512], b16, tag="ob4",
                                        bufs=2, name="ob4")
                        tb0 = 4 * iw + 2 * pi
                        for j in range(2):
                            g = 2 * pi + j
                            for mc in range(2):
                                pso = scratch()
                                nc.tensor.matmul(
                                    pso, yT[:, g],
                                    wo_sb[:, mc * 512:(mc + 1) * 512],
                                    start=True, stop=True,
                                )
                                drain_copy(ob4[:, j, mc], pso)
                            if tail:
                                tb = tb0 + j
                                dstj = out_d[tb * P:(tb + 1) * P, :]\
                                    .rearrange("p (mc i) -> p mc i", mc=2)
                                nc.sync.dma_start(dstj, ob4[:, j])
                        if not tail:
                            dst = out_d[tb0 * P:(tb0 + 2) * P, :].rearrange(
                                "(j p) (mc i) -> p j mc i", j=2, mc=2)
                            nc.sync.dma_start(dst, ob4)
                    outs.append(one)
                return outs

            def win512(iw, parts=(), oproj=(), last=False, head=()):
                """512-wide attention window over i in [512iw, 512iw+512),
                one-jc software-pipelined. Fillers run in the slack of the
                jc loop: qkv chunk micro-parts from jc>=1, the o-projection
                of window iw-2 from jc>=3."""
                isl = slice(iw * 512, (iw + 1) * 512)
                njc = 4 * iw + 4
                parts = list(parts)
                oproj = list(oproj)
                psy = [psp.tile([P, 4, P], f32, tag=f"psy{h}",
                                name="psy") for h in range(HPC)]

                def score_exp(jc, isl_, oiw):
                    eng = sched[(oiw, jc)]
                    m = jc - 4 * oiw if jc >= 4 * oiw else None
                    import os as _o
                    lo = m * P if (m is not None and _o.environ.get(
                        "DIAG_SCORE") == "1") else 0
                    jsl = slice(jc * P, (jc + 1) * P)
                    ps2 = psp.tile([P, 2, 512], f32, tag="pss2", bufs=2,
                                   name="ps2")
                    for h in range(HPC):
                        hb = D * h
                        nc.tensor.matmul(
                            ps2[:, h, lo:],
                            kT[hb:hb + D, jsl],
                            qT[hb:hb + D, isl_.start + lo:isl_.stop],
                            start=True, stop=True)
                    att2 = attp.tile([P, 2, 512], b16, tag="att2",
                                     name="att2", bufs=10)
                    if m is None:
                        if _o.environ.get("EXPSPLIT") == "1":
                            nc.scalar.activation(att2[:, 0], ps2[:, 0], EXP,
                                                 scale=SCALE)
                            nc.vector.tensor_scalar(
                                att2[:, 1].bitcast(i16), ps2[:, 1],
                                AS, B0, MUL, ADD)
                        elif eng == "a":
                            nc.scalar.activation(att2, ps2, EXP, scale=SCALE)
                        else:
                            nc.vector.tensor_scalar(att2[:].bitcast(i16),
                                                    ps2, AS, B0, MUL, ADD)
                    else:
                        lo = m * P
                        if eng == "am":
                            nc.scalar.activation(att2[:, :, lo:],
                                                 ps2[:, :, lo:], EXP,
                                                 scale=SCALE)
                            for h in range(HPC):
                                nc.vector.tensor_tensor(
                                    att2[:, h, lo:], att2[:, h, lo:],
                                    msk_sb[:, m, lo:], MUL)
                        else:
                            for h in range(HPC):
                                nc.vector.scalar_tensor_tensor(
                                    att2[:, h, lo:].bitcast(i16),
                                    ps2[:, h, lo:], AS,
                                    mskb_sb[:, m, lo:], MUL, ADD)
                    return att2, m

                def emit_attv(p):
                    jc0, a2, m0 = p
                    for h in range(HPC):
                        va = vaug[:, jc0, 65 * h:65 * h + 65]
                        for qs in range(4):
                            if m0 is not None and qs < m0:
                                continue
                            nc.tensor.matmul(
                                psy[h][:, qs, 0:65],
                                a2[:, h, qs * P:(qs + 1) * P], va,
                                start=(jc0 == 0 and qs == 0),
                                stop=(jc0 == njc - 1 and qs == 3))

                pend = list(head)
                for jc in range(len(head), njc):
                    att2, m = score_exp(jc, isl, iw)
                    if len(pend) == int(__import__("os").environ.get(
                            "PEND", "2")):
                        emit_attv(pend.pop(0))
                    pend.append((jc, att2, m))
                    if jc < njc - 3:
                        if jc >= 1 and parts:
                            parts.pop(0)()
                        elif jc >= 3 and oproj and (
                                jc % 2 == 1
                                or 3 * len(oproj) > njc - 3 - jc):
                            oproj.pop(0)()
                nxt = []
                if not last and iw > 0:
                    # pre-compute the next window's first score/exp blocks
                    # so PE has work while this window's tail drains
                    isl_n = slice((iw + 1) * 512, (iw + 2) * 512)
                    import os as _o6
                    for jc_n in range(int(_o6.environ.get("NXT", "3"))):
                        a2n, mn = score_exp(jc_n, isl_n, iw + 1)
                        nxt.append((jc_n, a2n, mn))
                for p in pend:
                    emit_attv(p)
                recs = []
                for h in range(HPC):
                    rec = smallp.tile([P, 4], f32, tag=f"rec{h}", bufs=3,
                                      name="rec")
                    nc.vector.reciprocal(rec, psy[h][:, :, 64])
                    recs.append(rec)
                yT = ropep.tile([P, 4, P], b16, tag="yt", bufs=3, name="yT")

                def norm_half(q0):
                    def f():
                        ysb = attp.tile([P, 2, P], b16, tag=f"ysb{q0}",
                                        bufs=2, name="ysb")
                        for qs in (q0, q0 + 1):
                            for h in range(HPC):
                                dst = ysb[:, qs - q0, 64 * h:64 * h + 64]
                                srcp = psy[h][:, qs, 0:64]
                                import os as _o7
                                nrm = _o7.environ.get(
                                    "NORMENG", "alt" if last else "a")
                                if nrm == "a" or (nrm == "alt"
                                                  and (qs + h) % 2 == 0):
                                    nc.scalar.activation(
                                        dst, srcp, CPY,
                                        scale=recs[h][:, qs:qs + 1])
                                else:
                                    nc.vector.tensor_scalar(
                                        dst, srcp, recs[h][:, qs:qs + 1],
                                        None, MUL)
                        nc.sync.dma_start_transpose(
                            yT[:, q0:q0 + 2, :],
                            ysb[:].rearrange("p q d -> p (q d)"))
                    return f

                import os
                if os.environ.get("OLD_NORM") == "1":
                    ysb = attp.tile([P, 4, P], b16, tag="ysb0", bufs=2,
                                    name="ysb")
                    for qs in range(4):
                        for h in range(HPC):
                            dst = ysb[:, qs, 64 * h:64 * h + 64]
                            srcp = psy[h][:, qs, 0:64]
                            if (qs + h) % 2 == 0:
                                nc.scalar.activation(
                                    dst, srcp, CPY,
                                    scale=recs[h][:, qs:qs + 1])
                            else:
                                nc.vector.tensor_scalar(
                                    dst, srcp, recs[h][:, qs:qs + 1],
                                    None, MUL)
                    nc.sync.dma_start_transpose(
                        yT, ysb[:].rearrange("p q d -> p (q d)"))
                    norms = []
                else:
                    norms = [norm_half(0), norm_half(2)]
                if norms and (last or os.environ.get("NO_DEFER_NORM") == "1"):
                    for f in norms:
                        f()
                    norms = []
                for f in parts + oproj:   # leftovers
                    f()
                return oproj_fillers(iw, yT, tail=last), nxt, norms

            qkv_chunk(0)
            c1 = qkv_parts(1, nc.scalar)
            for f in c1[:4]:
                f()
            late_consts()

            pend_ops = {}
            hd = ()
            norms = []
            for iw in range(nw):
                if iw == 0:
                    parts = c1[4:] + (qkv_parts(2, nc.scalar)
                                      if nt > 2 else [])
                else:
                    tch = iw + 2
                    import os as _o5
                    _cpm = int(_o5.environ.get("CPMAX", "7"))
                    parts = qkv_parts(
                        tch, nc.scalar if tch <= _cpm else nc.vector) \
                        if tch < nt else []
                if iw == nw - 2:
                    oproj = pend_ops.pop(iw - 2, []) + pend_ops.pop(iw - 1,
                                                                    [])
                elif iw == nw - 1:
                    oproj = pend_ops.pop(iw - 1, [])
                else:
                    oproj = pend_ops.pop(iw - 2, [])
                ops, hd, norms = win512(iw, parts=norms + parts,
                                        oproj=oproj,
                                        last=(iw == nw - 1), head=hd)
                pend_ops[iw] = ops
            for f in pend_ops.pop(nw - 1):
                f()

    nc.compile()
    return nc


def _perm_deinterleave():
    """Row permutation for Wq/Wk: per head, even rows then odd rows."""
    perm = []
    for h in range(H):
        base = h * D
        perm += [base + 2 * k for k in range(D // 2)]
        perm += [base + 2 * k + 1 for k in range(D // 2)]
    return np.array(perm)


def make_core_inputs(x, freqs_cos, freqs_sin, Wq, Wk, Wv, Wo, t=T):
    """Host-side sharding/layout prep. Returns per-core input dicts."""
    x = np.asarray(x, np.float32).reshape(t, C)
    fc = np.asarray(freqs_cos, np.float32)
    fs = np.asarray(freqs_sin, np.float32)
    Wq = np.asarray(Wq, np.float32)
    Wk = np.asarray(Wk, np.float32)
    Wv = np.asarray(Wv, np.float32)
    Wo = np.asarray(Wo, np.float32)

    xt = np.ascontiguousarray(x.T).astype(bf16)                  # [C, t]
    perm = _perm_deinterleave()
    Wq_p, Wk_p = Wq[perm], Wk[perm]

    # rope factor tables in the de-interleaved [dd, t] layout
    kidx = np.arange(P) % 32
    sgn = np.where((np.arange(P) // 32) % 2 == 0, -1.0, 1.0).astype(np.float32)
    cosb = fc.T[kidx].astype(bf16)                               # [128, t]
    sinb = (fs.T[kidx] * sgn[:, None]).astype(bf16)

    # diagonal-tile causal masks: valid iff 128*m + j <= i
    jj = np.arange(P)[:, None, None]
    mm = np.arange(4)[None, :, None]
    ii = np.arange(512)[None, None, :]
    valid = (P * mm + jj) <= ii
    mask4 = valid.astype(bf16)
    maskb = np.where(valid, np.float32(B0), np.float32(MASKED))

    def pack_w(w):
        # [C, DD] -> [P, C//P, DD]: one contiguous 2KB run per partition
        return np.ascontiguousarray(
            w.T.reshape(C // P, P, DD).transpose(1, 0, 2)).astype(bf16)

    in_maps = []
    for c in range(NCORES):
        rows = slice(c * DD, (c + 1) * DD)
        in_maps.append({
            "xt": xt,
            "wq": pack_w(Wq_p[rows]),
            "wk": pack_w(Wk_p[rows]),
            "wv": pack_w(Wv[rows]),
            "wo": np.ascontiguousarray(Wo[:, rows].T).astype(bf16),
            "cosb": cosb,
            "sinb": sinb,
            "mask4": mask4,
            "maskb": maskb,
        })
    return in_maps


def run(inputs, trace=False):
    """Compile once, run on 8 cores, host-sum partials."""
    import sys
    if "/opt/trn_rl_repo" not in sys.path:
        sys.path.insert(0, "/opt/trn_rl_repo")
    from concourse.bass_utils import run_bass_kernel_spmd

    if "nc" not in _nc_cache:
        _nc_cache["nc"] = _build_nc()
    nc = _nc_cache["nc"]

    in_maps = make_core_inputs(**inputs)
    res = run_bass_kernel_spmd(nc, in_maps, core_ids=list(range(NCORES)),
                               trace=trace)
    out = np.zeros((T, C), np.float64)
    for r in res.results:
        out += r["opart"].astype(np.float64)
    return out.astype(np.float32).reshape(1, T, C), res


def kernel(**inputs):
    import sys
    if "/opt/trn_rl_repo" not in sys.path:
        sys.path.insert(0, "/opt/trn_rl_repo")
    out, _ = run(inputs)
    return out


# revision 5
# speedup vs baseline: 1.0139x; 1.0007x over previous
"""Causal self-attention with RoPE, tensor-parallel over heads on 8 trn2 cores.

Reference computation (B=1, T=4096, C=1024, h=16, d=64, fp32):
    q/k/v = x @ W{q,k,v}^T ; rope(q), rope(k) ; causal softmax(q k^T / 8) v ; @ Wo^T

Sharding: 2 heads per core (tensor parallel). Each core reads the full x
(transposed + bf16 on host) and its slice of Wq/Wk/Wv (column-parallel) and
Wo (row-parallel). Cores emit partial o-projections; the host sums them.

Device-side layout choices (v2):
  - qT/kT [dhead(=128 both heads) x T] with the head dim de-interleaved
    (rope real parts in partitions 0-31 / 64-95, imag in 32-63 / 96-127) so
    rope's pair-swap is a partition-block swap done by 4 small DMAs.
  - scores are computed transposed: sT[j, i] = sum_d kT[d,j] qT[d,i].
  - exp runs on ACT (hw Exp) or DVE (bf16 Schraudolph: bitcast(int16(
    x*A + B)), ~1% softmax-weight ripple that largely cancels between the
    softmax numerator and denominator); diagonal blocks fold the causal
    mask into a DVE scalar_tensor_tensor with a +B / -20000 tile (masked
    lanes land at ~1e-37 ~ 0), or run ACT exp + 0/1 mask multiply,
    restricted to the valid column range.  A static schedule balances the
    two engines.
  - att@v is computed with att as the *stationary* operand ([128k x 128q]
    blocks) and the ones-augmented v [128k x 65] moving, 65 PE cycles per
    block; the 4 query sub-blocks of a head accumulate into one psum bank
    as a single lazily-zeroed accumulation group (row 64 = softmax
    denominator); fully-masked diagonal sub-blocks are skipped.
  - normalize: per-query reciprocal of the denominators + scaled
    psum->sbuf copies per (qs, head); one batched XBAR dma-transpose per
    window rebuilds yT [dd x tok] for the o-projection (bf16), whose
    output rows go out in 256-row batched DMAs.
"""

import numpy as np
import ml_dtypes

bf16 = ml_dtypes.bfloat16

T, C, H, D = 4096, 1024, 16, 64
NCORES = 8
HPC = H // NCORES          # heads per core
DD = HPC * D               # per-core qkv features (=128)
P = 128

A0 = 128.0 / np.log(2.0)   # Schraudolph bf16: e^x ~ bitcast(int16(x*A0 + B0))
B0 = 127.0 * 128.0 - 7.0
SCALE = 0.125
MASKED = -20000.0

_nc_cache = {}


def _exp_schedule(nw):
    """(iw, jc) -> 'a' | 'd' (interior) | 'am' | 'dm' (diagonal)."""
    import os
    sched = {}
    k = dk = 0
    pat = tuple(os.environ.get("EXP_PAT", "adad"))
    dpat = tuple((os.environ.get("DIAG_PAT", "damd") + "m")[i] + "m"
                 for i in range(4)) if False else None
    dpat = tuple(c + "m" for c in os.environ.get("DIAG_PAT", "aada"))
    if os.environ.get("EXP_ALL") == "a":
        pat, dpat = ("a",), ("am",)
    elif os.environ.get("EXP_ALL") == "d":
        pat, dpat = ("d",), ("dm",)
    for iw in range(nw):
        njc = 4 * iw + 4
        for jc in range(njc):
            if jc >= 4 * iw:
                sched[(iw, jc)] = dpat[(jc - 4 * iw) % len(dpat)]
                dk += 1
            else:
                sched[(iw, jc)] = pat[k % len(pat)]
                k += 1
    return sched


def _build_nc(t=T):
    import concourse.bass as bass
    import concourse.tile as tile
    import concourse.mybir as mybir
    from concourse import bacc

    f32 = mybir.dt.float32
    b16 = mybir.dt.bfloat16
    i16 = mybir.dt.int16
    MUL = mybir.AluOpType.mult
    ADD = mybir.AluOpType.add
    EXP = mybir.ActivationFunctionType.Exp
    CPY = mybir.ActivationFunctionType.Copy

    nt = t // 512            # qkv t-chunks
    nw = t // 512            # attention query windows
    njb = t // P             # key blocks
    AS = SCALE * A0

    sched = _exp_schedule(nw)

    nc = bacc.Bacc("TRN2")

    xt_d = nc.dram_tensor("xt", [C, t], b16, kind="ExternalInput")
    # weights host-prepacked to [P, C//P, DD]: one contiguous 2KB run per
    # partition (full-rate DMA descriptors)
    wq_d = nc.dram_tensor("wq", [P, C // P, DD], b16, kind="ExternalInput")
    wk_d = nc.dram_tensor("wk", [P, C // P, DD], b16, kind="ExternalInput")
    wv_d = nc.dram_tensor("wv", [P, C // P, DD], b16, kind="ExternalInput")
    wo_d = nc.dram_tensor("wo", [DD, C], b16, kind="ExternalInput")
    cos_d = nc.dram_tensor("cosb", [P, t], b16, kind="ExternalInput")
    sin_d = nc.dram_tensor("sinb", [P, t], b16, kind="ExternalInput")
    msk_d = nc.dram_tensor("mask4", [P, 4, 512], b16, kind="ExternalInput")
    mskb_d = nc.dram_tensor("maskb", [P, 4, 512], f32, kind="ExternalInput")
    out_d = nc.dram_tensor("opart", [t, C], b16, kind="ExternalOutput")

    with tile.TileContext(nc) as tc:
        with (
            tc.tile_pool(name="const", bufs=1) as constp,
            tc.tile_pool(name="xload", bufs=3) as xload,
            tc.tile_pool(name="rope", bufs=3) as ropep,
            tc.tile_pool(name="att", bufs=4) as attp,
            tc.tile_pool(name="small", bufs=4) as smallp,
            tc.tile_pool(name="ps", bufs=1, space="PSUM") as psp,
        ):
            # ---- constants / persistent tensors. DMA issue order chosen so
            # the first qkv matmuls (wq + x chunk 0) start ASAP.
            xt_pre = {}

            def load_xt(tch):
                # two half-loads so small latency-critical DMAs (rope swaps,
                # XBAR transposes) can slip between them on the DMA engines
                tsl = slice(tch * 512, (tch + 1) * 512)
                xt = xload.tile([P, C // P, 512], b16, name="xt", tag="xt")
                xv = xt_d[:].rearrange("(co p) t -> p co t", p=P)[:, :, tsl]
                import os as _o8
                if _o8.environ.get("XTSPLIT", "0") == "1":
                    nc.sync.dma_start(xt[:, 0:4], xv[:, 0:4])
                    nc.sync.dma_start(xt[:, 4:8], xv[:, 4:8])
                else:
                    nc.sync.dma_start(xt, xv)
                xt_pre[tch] = xt

            wq_sb = constp.tile([P, C // P, DD], b16)
            nc.sync.dma_start(wq_sb, wq_d[:])
            # x chunk 0 in four slices so the first qkv matmuls start sooner
            xt0 = xload.tile([P, C // P, 512], b16, name="xt0")
            xt_view = xt_d[:].rearrange("(co p) t -> p co t", p=P)
            for c4 in range(4):
                nc.sync.dma_start(xt0[:, 2 * c4:2 * c4 + 2],
                                  xt_view[:, 2 * c4:2 * c4 + 2, 0:512])
            xt_pre[0] = xt0
            wk_sb = constp.tile([P, C // P, DD], b16)
            nc.sync.dma_start(wk_sb, wk_d[:])
            load_xt(1)
            wv_sb = constp.tile([P, C // P, DD], b16)
            nc.sync.dma_start(wv_sb, wv_d[:])
            # rope tables: first two chunks' columns now, tails late
            cos_sb = constp.tile([P, t], b16)
            nc.sync.dma_start(cos_sb[:, 0:1024], cos_d[:, 0:1024])
            sin_sb = constp.tile([P, t], b16)
            nc.sync.dma_start(sin_sb[:, 0:1024], sin_d[:, 0:1024])
            # the first window's diagonal needs maskb[0] immediately
            mskb_sb = constp.tile([P, 4, 512], f32)
            nc.sync.dma_start(mskb_sb[:, 0:1], mskb_d[:, 0:1])
            msk_sb = constp.tile([P, 4, 512], b16)
            nc.sync.dma_start(msk_sb[:, 0:1], msk_d[:, 0:1])
            wo_sb = constp.tile([DD, C], b16)

            def late_consts():
                nc.sync.dma_start(mskb_sb[:, 1:4], mskb_d[:, 1:4])
                nc.sync.dma_start(msk_sb[:, 1:4], msk_d[:, 1:4])
                if t > 1024:
                    nc.sync.dma_start(cos_sb[:, 1024:], cos_d[:, 1024:])
                    nc.sync.dma_start(sin_sb[:, 1024:], sin_d[:, 1024:])
                nc.sync.dma_start(wo_sb, wo_d[:])

            qT = constp.tile([P, t], b16)   # rope'd q, both heads
            kT = constp.tile([P, t], b16)
            # v in natural layout per 128-block, +ones cols at 64 and 129
            vaug = constp.tile([P, njb, 2 * D + 2], b16)
            nc.vector.memset(vaug[:, :, D], 1.0)
            nc.vector.memset(vaug[:, :, 2 * D + 1], 1.0)

            # PSUM budget (8 banks): pss2 4KB x3 (scores, triple-buffered;
            # qkv and o-proj psums borrow half-slots from the same rotation)
            # + psy0/psy1 2KB x1 (att@v + denominator accumulators).
            def scratch():
                return psp.tile([P, 512], f32, tag="scratch", bufs=2,
                                name="scr")

            ob_k = {"k": 0}

            def drain_copy(dst, src_):
                """psum->sbuf copy alternating ACT/DVE."""
                ob_k["k"] += 1
                import os as _o4
                pat4 = _o4.environ.get("DRAIN", "ad")
                if pat4[ob_k["k"] % len(pat4)] == "a":
                    nc.scalar.copy(dst, src_)
                else:
                    nc.vector.tensor_copy(dst, src_)

            def qkv_parts(tch, cp):
                """Six micro-closures (~0.85us of PE each) spread through an
                attention window's PE slack. cp = engine for the psum->sbuf
                copies (ACT before the windows saturate it, DVE after)."""
                tsl = slice(tch * 512, (tch + 1) * 512)
                st = {}
                import os as _o3
                reng = nc.gpsimd if (
                    _o3.environ.get("GPSROPE", "1") == "1" and tch >= int(_o3.environ.get("GPSROPE_MIN", "2"))) \
                    else nc.vector

                def copy(dst, src_):
                    if cp is nc.scalar:
                        nc.scalar.copy(dst, src_)
                    else:
                        cp.tensor_copy(dst, src_)

                def mm_half(name, w_sb, half):
                    if name == "q" and half == 0:
                        st["xt"] = xt_pre.pop(tch)
                        st["q"] = scratch()
                    if name == "k" and half == 0:
                        if tch + 2 < nt and tch + 2 not in xt_pre:
                            load_xt(tch + 2)
                    if half == 0 and name != "q":
                        st[name] = scratch()
                    ps = st[name]
                    for ci in range(4 * half, 4 * half + 4):
                        nc.tensor.matmul(
                            ps, w_sb[:, ci], st["xt"][:, ci],
                            start=(ci == 0), stop=(ci == C // P - 1),
                        )

                def q1():
                    mm_half("q", wq_sb, 0)

                def q2():
                    mm_half("q", wq_sb, 1)
                    # q and k share one [128, 2, 512] tile; the rope
                    # pair-swap is partition-block DMAs issued right after
                    # each tensor's psum copy so the swap latency overlaps
                    # the other tensor's matmuls
                    st["qf2"] = ropep.tile([P, 2, 512], b16, tag="qf2",
                                           name="qf2")
                    st["sw2"] = ropep.tile([P, 2, 512], b16, tag="sw2",
                                           name="sw2")
                    copy(st["qf2"][:, 0], st["q"])
                    import os as _o2
                    if _o2.environ.get("EARLYSWAP") == "1":
                        qf2, sw2 = st["qf2"], st["sw2"]
                        nc.sync.dma_start(sw2[0:32, 0], qf2[32:64, 0])
                        nc.sync.dma_start(sw2[32:64, 0], qf2[0:32, 0])
                        nc.sync.dma_start(sw2[64:96, 0], qf2[96:128, 0])
                        nc.sync.dma_start(sw2[96:128, 0], qf2[64:96, 0])

                def k1():
                    mm_half("k", wk_sb, 0)

                def k2():
                    mm_half("k", wk_sb, 1)
                    qf2 = st["qf2"]
                    sw2 = st["sw2"]
                    copy(qf2[:, 1], st["k"])
                    import os as _o2
                    if _o2.environ.get("EARLYSWAP") == "1":
                        nc.sync.dma_start(sw2[0:32, 1], qf2[32:64, 1])
                        nc.sync.dma_start(sw2[32:64, 1], qf2[0:32, 1])
                        nc.sync.dma_start(sw2[64:96, 1], qf2[96:128, 1])
                        nc.sync.dma_start(sw2[96:128, 1], qf2[64:96, 1])
                    else:
                        nc.sync.dma_start(sw2[0:32], qf2[32:64])
                        nc.sync.dma_start(sw2[32:64], qf2[0:32])
                        nc.sync.dma_start(sw2[64:96], qf2[96:128])
                        nc.sync.dma_start(sw2[96:128], qf2[64:96])
                    for i, name in enumerate(("q", "k")):
                        tl = ropep.tile([P, 512], b16, tag=f"t1_{name}",
                                        name="t1")
                        reng.tensor_tensor(tl, qf2[:, i],
                                           cos_sb[:, tsl], MUL)
                        st[f"t1{name}"] = tl

                def v1():
                    t2q = ropep.tile([P, 512], b16, tag="t2_q", name="t2")
                    reng.tensor_tensor(t2q, st["sw2"][:, 0],
                                       sin_sb[:, tsl], MUL)
                    reng.tensor_add(qT[:, tsl], st["t1q"], t2q)
                    mm_half("v", wv_sb, 0)

                def v2():
                    mm_half("v", wv_sb, 1)
                    t2k = ropep.tile([P, 512], b16, tag="t2_k", name="t2")
                    reng.tensor_tensor(t2k, st["sw2"][:, 1],
                                       sin_sb[:, tsl], MUL)
                    reng.tensor_add(kT[:, tsl], st["t1k"], t2k)
                    # v: psum->sbuf copy, XBAR dma-transpose to dense [t, d]
                    # blocks, two narrow gpsimd copies around the ones column
                    vt = ropep.tile([P, 512], b16, tag="vt", name="vt")
                    copy(vt, st["v"])
                    dense = ropep.tile([P, 4, P], b16, tag="vdense",
                                       name="vdense")
                    nc.sync.dma_start_transpose(dense, vt[:])
                    g0 = tch * 4
                    nc.gpsimd.tensor_copy(vaug[:, g0:g0 + 4, 0:D],
                                          dense[:, :, 0:D])
                    nc.gpsimd.tensor_copy(vaug[:, g0:g0 + 4, D + 1:2 * D + 1],
                                          dense[:, :, D:2 * D])

                return [q1, q2, k1, k2, v1, v2]

            def qkv_chunk(tch):
                for f in qkv_parts(tch, nc.scalar):
                    f()

            def oproj_fillers(iw, yT, tail=False):
                """o-projection for two 256-row blocks, one batched output
                DMA each. Stationary yT blocks come from the window's XBAR
                transpose."""
                outs = []
                for pi in range(2):
                    def one(pi=pi):
                        ob4 = attp.tile([P, 2, 2, 512], b16, tag="ob4",
                                        bufs=2, name="ob4")
                        tb0 = 4 * iw + 2 * pi
                        for j in range(2):
                            g = 2 * pi + j
                            for mc in range(2):
                                pso = scratch()
                                nc.tensor.matmul(
                                    pso, yT[:, g],
                                    wo_sb[:, mc * 512:(mc + 1) * 512],
                                    start=True, stop=True,
                                )
                                drain_copy(ob4[:, j, mc], pso)
                            if tail:
                                tb = tb0 + j
                                dstj = out_d[tb * P:(tb + 1) * P, :]\
                                    .rearrange("p (mc i) -> p mc i", mc=2)
                                nc.sync.dma_start(dstj, ob4[:, j])
                        if not tail:
                            dst = out_d[tb0 * P:(tb0 + 2) * P, :].rearrange(
                                "(j p) (mc i) -> p j mc i", j=2, mc=2)
                            nc.sync.dma_start(dst, ob4)
                    outs.append(one)
                return outs

            def win512(iw, parts=(), oproj=(), last=False, head=()):
                """512-wide attention window over i in [512iw, 512iw+512),
                one-jc software-pipelined. Fillers run in the slack of the
                jc loop: qkv chunk micro-parts from jc>=1, the o-projection
                of window iw-2 from jc>=3."""
                isl = slice(iw * 512, (iw + 1) * 512)
                njc = 4 * iw + 4
                parts = list(parts)
                oproj = list(oproj)
                psy = [psp.tile([P, 4, P], f32, tag=f"psy{h}",
                                name="psy") for h in range(HPC)]

                def score_exp(jc, isl_, oiw):
                    eng = sched[(oiw, jc)]
                    m = jc - 4 * oiw if jc >= 4 * oiw else None
                    import os as _o
                    lo = m * P if (m is not None and _o.environ.get(
                        "DIAG_SCORE") == "1") else 0
                    jsl = slice(jc * P, (jc + 1) * P)
                    ps2 = psp.tile([P, 2, 512], f32, tag="pss2", bufs=2,
                                   name="ps2")
                    for h in range(HPC):
                        hb = D * h
                        nc.tensor.matmul(
                            ps2[:, h, lo:],
                            kT[hb:hb + D, jsl],
                            qT[hb:hb + D, isl_.start + lo:isl_.stop],
                            start=True, stop=True)
                    att2 = attp.tile([P, 2, 512], b16, tag="att2",
                                     name="att2", bufs=10)
                    if m is None:
                        if _o.environ.get("EXPSPLIT") == "1":
                            nc.scalar.activation(att2[:, 0], ps2[:, 0], EXP,
                                                 scale=SCALE)
                            nc.vector.tensor_scalar(
                                att2[:, 1].bitcast(i16), ps2[:, 1],
                                AS, B0, MUL, ADD)
                        elif _o.environ.get("EXPSPLIT") == "2":
                            if eng == "a":
                                nc.scalar.activation(att2[:, 0], ps2[:, 0],
                                                     EXP, scale=SCALE)
                                nc.scalar.activation(att2[:, 1], ps2[:, 1],
                                                     EXP, scale=SCALE)
                            else:
                                for hh in range(2):
                                    nc.vector.tensor_scalar(
                                        att2[:, hh].bitcast(i16),
                                        ps2[:, hh], AS, B0, MUL, ADD)
                        elif eng == "a":
                            nc.scalar.activation(att2, ps2, EXP, scale=SCALE)
                        else:
                            nc.vector.tensor_scalar(att2[:].bitcast(i16),
                                                    ps2, AS, B0, MUL, ADD)
                    else:
                        lo = m * P
                        if eng == "am":
                            nc.scalar.activation(att2[:, :, lo:],
                                                 ps2[:, :, lo:], EXP,
                                                 scale=SCALE)
                            for h in range(HPC):
                                nc.vector.tensor_tensor(
                                    att2[:, h, lo:], att2[:, h, lo:],
                                    msk_sb[:, m, lo:], MUL)
                        else:
                            for h in range(HPC):
                                nc.vector.scalar_tensor_tensor(
                                    att2[:, h, lo:].bitcast(i16),
                                    ps2[:, h, lo:], AS,
                                    mskb_sb[:, m, lo:], MUL, ADD)
                    return att2, m

                def emit_attv(p):
                    jc0, a2, m0 = p
                    for h in range(HPC):
                        va = vaug[:, jc0, 65 * h:65 * h + 65]
                        for qs in range(4):
                            if m0 is not None and qs < m0:
                                continue
                            nc.tensor.matmul(
                                psy[h][:, qs, 0:65],
                                a2[:, h, qs * P:(qs + 1) * P], va,
                                start=(jc0 == 0 and qs == 0),
                                stop=(jc0 == njc - 1 and qs == 3))

                pend = list(head)
                for jc in range(len(head), njc):
                    att2, m = score_exp(jc, isl, iw)
                    if len(pend) == int(__import__("os").environ.get(
                            "PEND", "2")):
                        emit_attv(pend.pop(0))
                    pend.append((jc, att2, m))
                    if jc < njc - 3:
                        if jc >= 1 and parts:
                            parts.pop(0)()
                        elif jc >= 3 and oproj and (
                                jc % 2 == 1
                                or 3 * len(oproj) > njc - 3 - jc):
                            oproj.pop(0)()
                nxt = []
                if not last and iw > 0:
                    # pre-compute the next window's first score/exp blocks
                    # so PE has work while this window's tail drains
                    isl_n = slice((iw + 1) * 512, (iw + 2) * 512)
                    import os as _o6
                    _nxt = int(_o6.environ.get("NXT", "3"))
                    if iw <= int(_o6.environ.get("NXT4MAX", "-1")):
                        _nxt = 4
                    for jc_n in range(_nxt):
                        a2n, mn = score_exp(jc_n, isl_n, iw + 1)
                        nxt.append((jc_n, a2n, mn))
                for p in pend:
                    emit_attv(p)
                recs = []
                for h in range(HPC):
                    rec = smallp.tile([P, 4], f32, tag=f"rec{h}", bufs=3,
                                      name="rec")
                    nc.vector.reciprocal(rec, psy[h][:, :, 64])
                    recs.append(rec)
                yT = ropep.tile([P, 4, P], b16, tag="yt", bufs=3, name="yT")

                def norm_half(q0):
                    def f():
                        ysb = attp.tile([P, 2, P], b16, tag=f"ysb{q0}",
                                        bufs=2, name="ysb")
                        for qs in (q0, q0 + 1):
                            for h in range(HPC):
                                dst = ysb[:, qs - q0, 64 * h:64 * h + 64]
                                srcp = psy[h][:, qs, 0:64]
                                import os as _o7
                                nrm = _o7.environ.get(
                                    "NORMENG", "alt" if last else "a")
                                if nrm == "a" or (nrm == "alt"
                                                  and (qs + h) % 2 == 0):
                                    nc.scalar.activation(
                                        dst, srcp, CPY,
                                        scale=recs[h][:, qs:qs + 1])
                                else:
                                    nc.vector.tensor_scalar(
                                        dst, srcp, recs[h][:, qs:qs + 1],
                                        None, MUL)
                        nc.sync.dma_start_transpose(
                            yT[:, q0:q0 + 2, :],
                            ysb[:].rearrange("p q d -> p (q d)"))
                    return f

                import os
                if os.environ.get("OLD_NORM") == "1":
                    ysb = attp.tile([P, 4, P], b16, tag="ysb0", bufs=2,
                                    name="ysb")
                    for qs in range(4):
                        for h in range(HPC):
                            dst = ysb[:, qs, 64 * h:64 * h + 64]
                            srcp = psy[h][:, qs, 0:64]
                            if (qs + h) % 2 == 0:
                                nc.scalar.activation(
                                    dst, srcp, CPY,
                                    scale=recs[h][:, qs:qs + 1])
                            else:
                                nc.vector.tensor_scalar(
                                    dst, srcp, recs[h][:, qs:qs + 1],
                                    None, MUL)
                    nc.sync.dma_start_transpose(
                        yT, ysb[:].rearrange("p q d -> p (q d)"))
                    norms = []
                else:
                    norms = [norm_half(0), norm_half(2)]
                if norms and (last or os.environ.get("NO_DEFER_NORM") == "1"):
                    for f in norms:
                        f()
                    norms = []
                for f in parts + oproj:   # leftovers
                    f()
                return oproj_fillers(iw, yT, tail=last), nxt, norms

            qkv_chunk(0)
            c1 = qkv_parts(1, nc.scalar)
            for f in c1[:4]:
                f()
            late_consts()

            pend_ops = {}
            hd = ()
            norms = []
            for iw in range(nw):
                if iw == 0:
                    parts = c1[4:] + (qkv_parts(2, nc.scalar)
                                      if nt > 2 else [])
                else:
                    tch = iw + 2
                    import os as _o5
                    _cpm = int(_o5.environ.get("CPMAX", "7"))
                    parts = qkv_parts(
                        tch, nc.scalar if tch <= _cpm else nc.vector) \
                        if tch < nt else []
                if iw == nw - 2:
                    oproj = pend_ops.pop(iw - 2, []) + pend_ops.pop(iw - 1,
                                                                    [])
                elif iw == nw - 1:
                    oproj = pend_ops.pop(iw - 1, [])
                else:
                    oproj = pend_ops.pop(iw - 2, [])
                ops, hd, norms = win512(iw, parts=norms + parts,
                                        oproj=oproj,
                                        last=(iw == nw - 1), head=hd)
                pend_ops[iw] = ops
            for f in pend_ops.pop(nw - 1):
                f()

    nc.compile()
    return nc


def _perm_deinterleave():
    """Row permutation for Wq/Wk: per head, even rows then odd rows."""
    perm = []
    for h in range(H):
        base = h * D
        perm += [base + 2 * k for k in range(D // 2)]
        perm += [base + 2 * k + 1 for k in range(D // 2)]
    return np.array(perm)


def make_core_inputs(x, freqs_cos, freqs_sin, Wq, Wk, Wv, Wo, t=T):
    """Host-side sharding/layout prep. Returns per-core input dicts."""
    x = np.asarray(x, np.float32).reshape(t, C)
    fc = np.asarray(freqs_cos, np.float32)
    fs = np.asarray(freqs_sin, np.float32)
    Wq = np.asarray(Wq, np.float32)
    Wk = np.asarray(Wk, np.float32)
    Wv = np.asarray(Wv, np.float32)
    Wo = np.asarray(Wo, np.float32)

    xt = np.ascontiguousarray(x.T).astype(bf16)                  # [C, t]
    perm = _perm_deinterleave()
    Wq_p, Wk_p = Wq[perm], Wk[perm]

    # rope factor tables in the de-interleaved [dd, t] layout
    kidx = np.arange(P) % 32
    sgn = np.where((np.arange(P) // 32) % 2 == 0, -1.0, 1.0).astype(np.float32)
    cosb = fc.T[kidx].astype(bf16)                               # [128, t]
    sinb = (fs.T[kidx] * sgn[:, None]).astype(bf16)

    # diagonal-tile causal masks: valid iff 128*m + j <= i
    jj = np.arange(P)[:, None, None]
    mm = np.arange(4)[None, :, None]
    ii = np.arange(512)[None, None, :]
    valid = (P * mm + jj) <= ii
    mask4 = valid.astype(bf16)
    maskb = np.where(valid, np.float32(B0), np.float32(MASKED))

    def pack_w(w):
        # [C, DD] -> [P, C//P, DD]: one contiguous 2KB run per partition
        return np.ascontiguousarray(
            w.T.reshape(C // P, P, DD).transpose(1, 0, 2)).astype(bf16)

    in_maps = []
    for c in range(NCORES):
        rows = slice(c * DD, (c + 1) * DD)
        in_maps.append({
            "xt": xt,
            "wq": pack_w(Wq_p[rows]),
            "wk": pack_w(Wk_p[rows]),
            "wv": pack_w(Wv[rows]),
            "wo": np.ascontiguousarray(Wo[:, rows].T).astype(bf16),
            "cosb": cosb,
            "sinb": sinb,
            "mask4": mask4,
            "maskb": maskb,
        })
    return in_maps


def run(inputs, trace=False):
    """Compile once, run on 8 cores, host-sum partials."""
    import sys
    if "/opt/trn_rl_repo" not in sys.path:
        sys.path.insert(0, "/opt/trn_rl_repo")
    from concourse.bass_utils import run_bass_kernel_spmd

    if "nc" not in _nc_cache:
        _nc_cache["nc"] = _build_nc()
    nc = _nc_cache["nc"]

    in_maps = make_core_inputs(**inputs)
    res = run_bass_kernel_spmd(nc, in_maps, core_ids=list(range(NCORES)),
                               trace=trace)
    out = np.zeros((T, C), np.float64)
    for r in res.results:
        out += r["opart"].astype(np.float64)
    return out.astype(np.float32).reshape(1, T, C), res


def kernel(**inputs):
    import sys
    if "/opt/trn_rl_repo" not in sys.path:
        sys.path.insert(0, "/opt/trn_rl_repo")
    out, _ = run(inputs)
    return out


# revision 7
# speedup vs baseline: 1.0207x; 1.0067x over previous
"""Causal self-attention with RoPE, tensor-parallel over heads on 8 trn2 cores.

Reference computation (B=1, T=4096, C=1024, h=16, d=64, fp32):
    q/k/v = x @ W{q,k,v}^T ; rope(q), rope(k) ; causal softmax(q k^T / 8) v ; @ Wo^T

Sharding: 2 heads per core (tensor parallel). Each core reads the full x
(transposed + bf16 on host) and its slice of Wq/Wk/Wv (column-parallel) and
Wo (row-parallel). Cores emit partial o-projections; the host sums them.

Device-side layout choices (v2):
  - qT/kT [dhead(=128 both heads) x T] with the head dim de-interleaved
    (rope real parts in partitions 0-31 / 64-95, imag in 32-63 / 96-127) so
    rope's pair-swap is a partition-block swap done by 4 small DMAs.
  - scores are computed transposed: sT[j, i] = sum_d kT[d,j] qT[d,i].
  - exp runs on ACT (hw Exp) or DVE (bf16 Schraudolph: bitcast(int16(
    x*A + B)), ~1% softmax-weight ripple that largely cancels between the
    softmax numerator and denominator); diagonal blocks fold the causal
    mask into a DVE scalar_tensor_tensor with a +B / -20000 tile (masked
    lanes land at ~1e-37 ~ 0), or run ACT exp + 0/1 mask multiply,
    restricted to the valid column range.  A static schedule balances the
    two engines.
  - att@v is computed with att as the *stationary* operand ([128k x 128q]
    blocks) and the ones-augmented v [128k x 65] moving, 65 PE cycles per
    block; the 4 query sub-blocks of a head accumulate into one psum bank
    as a single lazily-zeroed accumulation group (row 64 = softmax
    denominator); fully-masked diagonal sub-blocks are skipped.
  - normalize: per-query reciprocal of the denominators + scaled
    psum->sbuf copies per (qs, head); one batched XBAR dma-transpose per
    window rebuilds yT [dd x tok] for the o-projection (bf16), whose
    output rows go out in 256-row batched DMAs.
"""

import numpy as np
import ml_dtypes

bf16 = ml_dtypes.bfloat16

T, C, H, D = 4096, 1024, 16, 64
NCORES = 8
HPC = H // NCORES          # heads per core
DD = HPC * D               # per-core qkv features (=128)
P = 128

A0 = 128.0 / np.log(2.0)   # Schraudolph bf16: e^x ~ bitcast(int16(x*A0 + B0))
B0 = 127.0 * 128.0 - 7.0
SCALE = 0.125
MASKED = -20000.0

_nc_cache = {}


def _exp_schedule(nw):
    """(iw, jc) -> 'a' | 'd' (interior) | 'am' | 'dm' (diagonal)."""
    import os
    sched = {}
    k = dk = 0
    pat = tuple(os.environ.get("EXP_PAT", "adad"))
    dpat = tuple(c + "m" for c in os.environ.get("DIAG_PAT", "aada"))
    pat_l = tuple(os.environ.get("EXP_PAT_L", os.environ.get(
        "EXP_PAT", "adad")))
    dpat_l = tuple(c + "m" for c in os.environ.get(
        "DIAG_PAT_L", "aaaa"))
    if os.environ.get("EXP_ALL") == "a":
        pat, dpat = ("a",), ("am",)
    elif os.environ.get("EXP_ALL") == "d":
        pat, dpat = ("d",), ("dm",)
    for iw in range(nw):
        njc = 4 * iw + 4
        p, dp = (pat, dpat) if iw < nw // 2 else (pat_l, dpat_l)
        for jc in range(njc):
            if jc >= 4 * iw:
                sched[(iw, jc)] = dp[(jc - 4 * iw) % len(dp)]
                dk += 1
            else:
                sched[(iw, jc)] = p[k % len(p)]
                k += 1
    return sched


def _build_nc(t=T):
    import concourse.bass as bass
    import concourse.tile as tile
    import concourse.mybir as mybir
    from concourse import bacc

    f32 = mybir.dt.float32
    b16 = mybir.dt.bfloat16
    i16 = mybir.dt.int16
    MUL = mybir.AluOpType.mult
    ADD = mybir.AluOpType.add
    EXP = mybir.ActivationFunctionType.Exp
    CPY = mybir.ActivationFunctionType.Copy

    nt = t // 512            # qkv t-chunks
    nw = t // 512            # attention query windows
    njb = t // P             # key blocks
    AS = SCALE * A0

    sched = _exp_schedule(nw)

    nc = bacc.Bacc("TRN2")

    xt_d = nc.dram_tensor("xt", [C, t], b16, kind="ExternalInput")
    # weights host-prepacked to [P, C//P, DD]: one contiguous 2KB run per
    # partition (full-rate DMA descriptors)
    wq_d = nc.dram_tensor("wq", [P, C // P, DD], b16, kind="ExternalInput")
    wk_d = nc.dram_tensor("wk", [P, C // P, DD], b16, kind="ExternalInput")
    wv_d = nc.dram_tensor("wv", [P, C // P, DD], b16, kind="ExternalInput")
    wo_d = nc.dram_tensor("wo", [DD, C], b16, kind="ExternalInput")
    cos_d = nc.dram_tensor("cosb", [P, t], b16, kind="ExternalInput")
    sin_d = nc.dram_tensor("sinb", [P, t], b16, kind="ExternalInput")
    msk_d = nc.dram_tensor("mask4", [P, 4, 512], b16, kind="ExternalInput")
    mskb_d = nc.dram_tensor("maskb", [P, 4, 512], f32, kind="ExternalInput")
    out_d = nc.dram_tensor("opart", [t, C], b16, kind="ExternalOutput")

    with tile.TileContext(nc) as tc:
        with (
            tc.tile_pool(name="const", bufs=1) as constp,
            tc.tile_pool(name="xload", bufs=3) as xload,
            tc.tile_pool(name="rope", bufs=3) as ropep,
            tc.tile_pool(name="att", bufs=4) as attp,
            tc.tile_pool(name="small", bufs=4) as smallp,
            tc.tile_pool(name="ps", bufs=1, space="PSUM") as psp,
        ):
            # ---- constants / persistent tensors. DMA issue order chosen so
            # the first qkv matmuls (wq + x chunk 0) start ASAP.
            xt_pre = {}

            def load_xt(tch):
                # two half-loads so small latency-critical DMAs (rope swaps,
                # XBAR transposes) can slip between them on the DMA engines
                tsl = slice(tch * 512, (tch + 1) * 512)
                xt = xload.tile([P, C // P, 512], b16, name="xt", tag="xt")
                xv = xt_d[:].rearrange("(co p) t -> p co t", p=P)[:, :, tsl]
                import os as _o8
                if _o8.environ.get("XTSPLIT", "0") == "1":
                    nc.sync.dma_start(xt[:, 0:4], xv[:, 0:4])
                    nc.sync.dma_start(xt[:, 4:8], xv[:, 4:8])
                else:
                    nc.sync.dma_start(xt, xv)
                xt_pre[tch] = xt

            wq_sb = constp.tile([P, C // P, DD], b16)
            nc.sync.dma_start(wq_sb, wq_d[:])
            # x chunk 0 in four slices so the first qkv matmuls start sooner
            xt0 = xload.tile([P, C // P, 512], b16, name="xt0")
            xt_view = xt_d[:].rearrange("(co p) t -> p co t", p=P)
            for c4 in range(4):
                nc.sync.dma_start(xt0[:, 2 * c4:2 * c4 + 2],
                                  xt_view[:, 2 * c4:2 * c4 + 2, 0:512])
            xt_pre[0] = xt0
            wk_sb = constp.tile([P, C // P, DD], b16)
            nc.sync.dma_start(wk_sb, wk_d[:])
            load_xt(1)
            wv_sb = constp.tile([P, C // P, DD], b16)
            nc.sync.dma_start(wv_sb, wv_d[:])
            # rope tables: first two chunks' columns now, tails late
            cos_sb = constp.tile([P, t], b16)
            nc.sync.dma_start(cos_sb[:, 0:1024], cos_d[:, 0:1024])
            sin_sb = constp.tile([P, t], b16)
            nc.sync.dma_start(sin_sb[:, 0:1024], sin_d[:, 0:1024])
            # the first window's diagonal needs maskb[0] immediately
            mskb_sb = constp.tile([P, 4, 512], f32)
            nc.sync.dma_start(mskb_sb[:, 0:1], mskb_d[:, 0:1])
            msk_sb = constp.tile([P, 4, 512], b16)
            nc.sync.dma_start(msk_sb[:, 0:1], msk_d[:, 0:1])
            wo_sb = constp.tile([DD, C], b16)

            def late_consts():
                nc.sync.dma_start(mskb_sb[:, 1:4], mskb_d[:, 1:4])
                nc.sync.dma_start(msk_sb[:, 1:4], msk_d[:, 1:4])
                if t > 1024:
                    nc.sync.dma_start(cos_sb[:, 1024:], cos_d[:, 1024:])
                    nc.sync.dma_start(sin_sb[:, 1024:], sin_d[:, 1024:])
                nc.sync.dma_start(wo_sb, wo_d[:])

            qT = constp.tile([P, t], b16)   # rope'd q, both heads
            kT = constp.tile([P, t], b16)
            # v in natural layout per 128-block, +ones cols at 64 and 129
            vaug = constp.tile([P, njb, 2 * D + 2], b16)
            nc.vector.memset(vaug[:, :, D], 1.0)
            nc.vector.memset(vaug[:, :, 2 * D + 1], 1.0)

            # PSUM budget (8 banks): pss2 4KB x3 (scores, triple-buffered;
            # qkv and o-proj psums borrow half-slots from the same rotation)
            # + psy0/psy1 2KB x1 (att@v + denominator accumulators).
            def scratch():
                return psp.tile([P, 512], f32, tag="scratch", bufs=2,
                                name="scr")

            ob_k = {"k": 0}

            def drain_copy(dst, src_):
                """psum->sbuf copy alternating ACT/DVE."""
                ob_k["k"] += 1
                import os as _o4
                pat4 = _o4.environ.get("DRAIN", "ad")
                if pat4[ob_k["k"] % len(pat4)] == "a":
                    nc.scalar.copy(dst, src_)
                else:
                    nc.vector.tensor_copy(dst, src_)

            def qkv_parts(tch, cp):
                """Six micro-closures (~0.85us of PE each) spread through an
                attention window's PE slack. cp = engine for the psum->sbuf
                copies (ACT before the windows saturate it, DVE after)."""
                tsl = slice(tch * 512, (tch + 1) * 512)
                st = {}
                import os as _o3
                reng = nc.gpsimd if (
                    _o3.environ.get("GPSROPE", "1") == "1" and tch >= int(_o3.environ.get("GPSROPE_MIN", "2"))) \
                    else nc.vector

                def copy(dst, src_):
                    if cp is nc.scalar:
                        nc.scalar.copy(dst, src_)
                    else:
                        cp.tensor_copy(dst, src_)

                def mm_half(name, w_sb, half):
                    if name == "q" and half == 0:
                        st["xt"] = xt_pre.pop(tch)
                        st["q"] = scratch()
                    if name == "k" and half == 0:
                        if tch + 2 < nt and tch + 2 not in xt_pre:
                            load_xt(tch + 2)
                    if half == 0 and name != "q":
                        st[name] = scratch()
                    ps = st[name]
                    for ci in range(4 * half, 4 * half + 4):
                        nc.tensor.matmul(
                            ps, w_sb[:, ci], st["xt"][:, ci],
                            start=(ci == 0), stop=(ci == C // P - 1),
                        )

                def q1():
                    mm_half("q", wq_sb, 0)

                def q2():
                    mm_half("q", wq_sb, 1)
                    # q and k share one [128, 2, 512] tile; the rope
                    # pair-swap is partition-block DMAs issued right after
                    # each tensor's psum copy so the swap latency overlaps
                    # the other tensor's matmuls
                    st["qf2"] = ropep.tile([P, 2, 512], b16, tag="qf2",
                                           name="qf2")
                    st["sw2"] = ropep.tile([P, 2, 512], b16, tag="sw2",
                                           name="sw2")
                    copy(st["qf2"][:, 0], st["q"])
                    import os as _o2
                    if _o2.environ.get("EARLYSWAP") == "1":
                        qf2, sw2 = st["qf2"], st["sw2"]
                        nc.sync.dma_start(sw2[0:32, 0], qf2[32:64, 0])
                        nc.sync.dma_start(sw2[32:64, 0], qf2[0:32, 0])
                        nc.sync.dma_start(sw2[64:96, 0], qf2[96:128, 0])
                        nc.sync.dma_start(sw2[96:128, 0], qf2[64:96, 0])

                def k1():
                    mm_half("k", wk_sb, 0)

                def k2():
                    mm_half("k", wk_sb, 1)
                    qf2 = st["qf2"]
                    sw2 = st["sw2"]
                    copy(qf2[:, 1], st["k"])
                    import os as _o2
                    if _o2.environ.get("EARLYSWAP") == "1":
                        nc.sync.dma_start(sw2[0:32, 1], qf2[32:64, 1])
                        nc.sync.dma_start(sw2[32:64, 1], qf2[0:32, 1])
                        nc.sync.dma_start(sw2[64:96, 1], qf2[96:128, 1])
                        nc.sync.dma_start(sw2[96:128, 1], qf2[64:96, 1])
                    else:
                        nc.sync.dma_start(sw2[0:32], qf2[32:64])
                        nc.sync.dma_start(sw2[32:64], qf2[0:32])
                        nc.sync.dma_start(sw2[64:96], qf2[96:128])
                        nc.sync.dma_start(sw2[96:128], qf2[64:96])
                    for i, name in enumerate(("q", "k")):
                        tl = ropep.tile([P, 512], b16, tag=f"t1_{name}",
                                        name="t1")
                        reng.tensor_tensor(tl, qf2[:, i],
                                           cos_sb[:, tsl], MUL)
                        st[f"t1{name}"] = tl

                def v1():
                    t2q = ropep.tile([P, 512], b16, tag="t2_q", name="t2")
                    reng.tensor_tensor(t2q, st["sw2"][:, 0],
                                       sin_sb[:, tsl], MUL)
                    reng.tensor_add(qT[:, tsl], st["t1q"], t2q)
                    mm_half("v", wv_sb, 0)

                def v2():
                    mm_half("v", wv_sb, 1)
                    t2k = ropep.tile([P, 512], b16, tag="t2_k", name="t2")
                    reng.tensor_tensor(t2k, st["sw2"][:, 1],
                                       sin_sb[:, tsl], MUL)
                    reng.tensor_add(kT[:, tsl], st["t1k"], t2k)
                    # v: psum->sbuf copy, XBAR dma-transpose to dense [t, d]
                    # blocks, two narrow gpsimd copies around the ones column
                    vt = ropep.tile([P, 512], b16, tag="vt", name="vt")
                    copy(vt, st["v"])
                    dense = ropep.tile([P, 4, P], b16, tag="vdense",
                                       name="vdense")
                    nc.sync.dma_start_transpose(dense, vt[:])
                    g0 = tch * 4
                    nc.gpsimd.tensor_copy(vaug[:, g0:g0 + 4, 0:D],
                                          dense[:, :, 0:D])
                    nc.gpsimd.tensor_copy(vaug[:, g0:g0 + 4, D + 1:2 * D + 1],
                                          dense[:, :, D:2 * D])

                return [q1, q2, k1, k2, v1, v2]

            def qkv_chunk(tch):
                for f in qkv_parts(tch, nc.scalar):
                    f()

            def oproj_fillers(iw, yT, tail=False):
                """o-projection for two 256-row blocks, one batched output
                DMA each. Stationary yT blocks come from the window's XBAR
                transpose."""
                outs = []
                for pi in range(2):
                    def one(pi=pi):
                        ob4 = attp.tile([P, 2, 2, 512], b16, tag="ob4",
                                        bufs=int(__import__("os").environ.get(
                                            "OB4B", "3")), name="ob4")
                        tb0 = 4 * iw + 2 * pi
                        for j in range(2):
                            g = 2 * pi + j
                            for mc in range(2):
                                pso = scratch()
                                nc.tensor.matmul(
                                    pso, yT[:, g],
                                    wo_sb[:, mc * 512:(mc + 1) * 512],
                                    start=True, stop=True,
                                )
                                drain_copy(ob4[:, j, mc], pso)
                            if tail:
                                tb = tb0 + j
                                dstj = out_d[tb * P:(tb + 1) * P, :]\
                                    .rearrange("p (mc i) -> p mc i", mc=2)
                                nc.sync.dma_start(dstj, ob4[:, j])
                        if not tail:
                            dst = out_d[tb0 * P:(tb0 + 2) * P, :].rearrange(
                                "(j p) (mc i) -> p j mc i", j=2, mc=2)
                            nc.sync.dma_start(dst, ob4)
                    outs.append(one)
                return outs

            def win512(iw, parts=(), oproj=(), last=False, head=()):
                """512-wide attention window over i in [512iw, 512iw+512),
                one-jc software-pipelined. Fillers run in the slack of the
                jc loop: qkv chunk micro-parts from jc>=1, the o-projection
                of window iw-2 from jc>=3."""
                isl = slice(iw * 512, (iw + 1) * 512)
                njc = 4 * iw + 4
                parts = list(parts)
                oproj = list(oproj)
                psy = [psp.tile([P, 4, P], f32, tag=f"psy{h}",
                                name="psy") for h in range(HPC)]

                def score_exp(jc, isl_, oiw):
                    eng = sched[(oiw, jc)]
                    m = jc - 4 * oiw if jc >= 4 * oiw else None
                    import os as _o
                    lo = m * P if (m is not None and _o.environ.get(
                        "DIAG_SCORE") == "1") else 0
                    jsl = slice(jc * P, (jc + 1) * P)
                    ps2 = psp.tile([P, 2, 512], f32, tag="pss2", bufs=2,
                                   name="ps2")
                    for h in range(HPC):
                        hb = D * h
                        nc.tensor.matmul(
                            ps2[:, h, lo:],
                            kT[hb:hb + D, jsl],
                            qT[hb:hb + D, isl_.start + lo:isl_.stop],
                            start=True, stop=True)
                    att2 = attp.tile([P, 2, 512], b16, tag="att2",
                                     name="att2", bufs=int(__import__("os").environ.get("A2B", "14")))
                    if m is None:
                        if _o.environ.get("EXPSPLIT") == "1":
                            nc.scalar.activation(att2[:, 0], ps2[:, 0], EXP,
                                                 scale=SCALE)
                            nc.vector.tensor_scalar(
                                att2[:, 1].bitcast(i16), ps2[:, 1],
                                AS, B0, MUL, ADD)
                        elif _o.environ.get("EXPSPLIT") == "2":
                            if eng == "a":
                                nc.scalar.activation(att2[:, 0], ps2[:, 0],
                                                     EXP, scale=SCALE)
                                nc.scalar.activation(att2[:, 1], ps2[:, 1],
                                                     EXP, scale=SCALE)
                            else:
                                for hh in range(2):
                                    nc.vector.tensor_scalar(
                                        att2[:, hh].bitcast(i16),
                                        ps2[:, hh], AS, B0, MUL, ADD)
                        elif eng == "a":
                            nc.scalar.activation(att2, ps2, EXP, scale=SCALE)
                        else:
                            nc.vector.tensor_scalar(att2[:].bitcast(i16),
                                                    ps2, AS, B0, MUL, ADD)
                    else:
                        lo = m * P
                        if eng == "am":
                            nc.scalar.activation(att2[:, :, lo:],
                                                 ps2[:, :, lo:], EXP,
                                                 scale=SCALE)
                            for h in range(HPC):
                                nc.vector.tensor_tensor(
                                    att2[:, h, lo:], att2[:, h, lo:],
                                    msk_sb[:, m, lo:], MUL)
                        else:
                            for h in range(HPC):
                                nc.vector.scalar_tensor_tensor(
                                    att2[:, h, lo:].bitcast(i16),
                                    ps2[:, h, lo:], AS,
                                    mskb_sb[:, m, lo:], MUL, ADD)
                    return att2, m

                def emit_attv(p):
                    jc0, a2, m0 = p
                    for h in range(HPC):
                        va = vaug[:, jc0, 65 * h:65 * h + 65]
                        for qs in range(4):
                            if m0 is not None and qs < m0:
                                continue
                            nc.tensor.matmul(
                                psy[h][:, qs, 0:65],
                                a2[:, h, qs * P:(qs + 1) * P], va,
                                start=(jc0 == 0 and qs == 0),
                                stop=(jc0 == njc - 1 and qs == 3))

                pend = list(head)
                for jc in range(len(head), njc):
                    att2, m = score_exp(jc, isl, iw)
                    if len(pend) == int(__import__("os").environ.get(
                            "PEND", "2")):
                        emit_attv(pend.pop(0))
                    pend.append((jc, att2, m))
                    if jc < njc - 3:
                        if jc >= 1 and parts:
                            parts.pop(0)()
                        elif jc >= 3 and oproj and (
                                jc % 2 == 1
                                or 3 * len(oproj) > njc - 3 - jc):
                            oproj.pop(0)()
                nxt = []
                if not last and iw > 0:
                    # pre-compute the next window's first score/exp blocks
                    # so PE has work while this window's tail drains
                    isl_n = slice((iw + 1) * 512, (iw + 2) * 512)
                    import os as _o6
                    _nxt = int(_o6.environ.get("NXT", "3"))
                    if iw <= int(_o6.environ.get("NXT4MAX", "-1")):
                        _nxt = 4
                    for jc_n in range(_nxt):
                        a2n, mn = score_exp(jc_n, isl_n, iw + 1)
                        nxt.append((jc_n, a2n, mn))
                for p in pend:
                    emit_attv(p)
                recs = []
                for h in range(HPC):
                    rec = smallp.tile([P, 4], f32, tag=f"rec{h}", bufs=3,
                                      name="rec")
                    nc.vector.reciprocal(rec, psy[h][:, :, 64])
                    recs.append(rec)
                yT = ropep.tile([P, 4, P], b16, tag="yt", bufs=int(__import__("os").environ.get("YTB", "3")), name="yT")

                def norm_half(q0):
                    def f():
                        ysb = attp.tile([P, 2, P], b16, tag=f"ysb{q0}",
                                        bufs=int(__import__("os").environ.get("YSBB", "2")), name="ysb")
                        for qs in (q0, q0 + 1):
                            for h in range(HPC):
                                dst = ysb[:, qs - q0, 64 * h:64 * h + 64]
                                srcp = psy[h][:, qs, 0:64]
                                import os as _o7
                                nrm = _o7.environ.get(
                                    "NORMENG", "alt" if last else "a")
                                if nrm == "a" or (nrm == "alt"
                                                  and (qs + h) % 2 == 0):
                                    nc.scalar.activation(
                                        dst, srcp, CPY,
                                        scale=recs[h][:, qs:qs + 1])
                                else:
                                    nc.vector.tensor_scalar(
                                        dst, srcp, recs[h][:, qs:qs + 1],
                                        None, MUL)
                        nc.sync.dma_start_transpose(
                            yT[:, q0:q0 + 2, :],
                            ysb[:].rearrange("p q d -> p (q d)"))
                    return f

                import os
                if os.environ.get("OLD_NORM") == "1":
                    ysb = attp.tile([P, 4, P], b16, tag="ysb0", bufs=2,
                                    name="ysb")
                    for qs in range(4):
                        for h in range(HPC):
                            dst = ysb[:, qs, 64 * h:64 * h + 64]
                            srcp = psy[h][:, qs, 0:64]
                            if (qs + h) % 2 == 0:
                                nc.scalar.activation(
                                    dst, srcp, CPY,
                                    scale=recs[h][:, qs:qs + 1])
                            else:
                                nc.vector.tensor_scalar(
                                    dst, srcp, recs[h][:, qs:qs + 1],
                                    None, MUL)
                    nc.sync.dma_start_transpose(
                        yT, ysb[:].rearrange("p q d -> p (q d)"))
                    norms = []
                else:
                    norms = [norm_half(0), norm_half(2)]
                if norms and (last or os.environ.get("NO_DEFER_NORM") == "1"):
                    for f in norms:
                        f()
                    norms = []
                for f in parts + oproj:   # leftovers
                    f()
                return oproj_fillers(iw, yT, tail=last), nxt, norms

            qkv_chunk(0)
            c1 = qkv_parts(1, nc.scalar)
            for f in c1[:4]:
                f()
            late_consts()

            pend_ops = {}
            hd = ()
            norms = []
            for iw in range(nw):
                if iw == 0:
                    parts = c1[4:] + (qkv_parts(2, nc.scalar)
                                      if nt > 2 else [])
                else:
                    tch = iw + 2
                    import os as _o5
                    _cpm = int(_o5.environ.get("CPMAX", "7"))
                    parts = qkv_parts(
                        tch, nc.scalar if tch <= _cpm else nc.vector) \
                        if tch < nt else []
                if iw == nw - 2:
                    oproj = pend_ops.pop(iw - 2, []) + pend_ops.pop(iw - 1,
                                                                    [])
                elif iw == nw - 1:
                    oproj = pend_ops.pop(iw - 1, [])
                else:
                    oproj = pend_ops.pop(iw - 2, [])
                ops, hd, norms = win512(iw, parts=norms + parts,
                                        oproj=oproj,
                                        last=(iw == nw - 1), head=hd)
                pend_ops[iw] = ops
            for f in pend_ops.pop(nw - 1):
                f()

    nc.compile()
    return nc


def _perm_deinterleave():
    """Row permutation for Wq/Wk: per head, even rows then odd rows."""
    perm = []
    for h in range(H):
        base = h * D
        perm += [base + 2 * k for k in range(D // 2)]
        perm += [base + 2 * k + 1 for k in range(D // 2)]
    return np.array(perm)


def make_core_inputs(x, freqs_cos, freqs_sin, Wq, Wk, Wv, Wo, t=T):
    """Host-side sharding/layout prep. Returns per-core input dicts."""
    x = np.asarray(x, np.float32).reshape(t, C)
    fc = np.asarray(freqs_cos, np.float32)
    fs = np.asarray(freqs_sin, np.float32)
    Wq = np.asarray(Wq, np.float32)
    Wk = np.asarray(Wk, np.float32)
    Wv = np.asarray(Wv, np.float32)
    Wo = np.asarray(Wo, np.float32)

    xt = np.ascontiguousarray(x.T).astype(bf16)                  # [C, t]
    perm = _perm_deinterleave()
    Wq_p, Wk_p = Wq[perm], Wk[perm]

    # rope factor tables in the de-interleaved [dd, t] layout
    kidx = np.arange(P) % 32
    sgn = np.where((np.arange(P) // 32) % 2 == 0, -1.0, 1.0).astype(np.float32)
    cosb = fc.T[kidx].astype(bf16)                               # [128, t]
    sinb = (fs.T[kidx] * sgn[:, None]).astype(bf16)

    # diagonal-tile causal masks: valid iff 128*m + j <= i
    jj = np.arange(P)[:, None, None]
    mm = np.arange(4)[None, :, None]
    ii = np.arange(512)[None, None, :]
    valid = (P * mm + jj) <= ii
    mask4 = valid.astype(bf16)
    maskb = np.where(valid, np.float32(B0), np.float32(MASKED))

    def pack_w(w):
        # [C, DD] -> [P, C//P, DD]: one contiguous 2KB run per partition
        return np.ascontiguousarray(
            w.T.reshape(C // P, P, DD).transpose(1, 0, 2)).astype(bf16)

    in_maps = []
    for c in range(NCORES):
        rows = slice(c * DD, (c + 1) * DD)
        in_maps.append({
            "xt": xt,
            "wq": pack_w(Wq_p[rows]),
            "wk": pack_w(Wk_p[rows]),
            "wv": pack_w(Wv[rows]),
            "wo": np.ascontiguousarray(Wo[:, rows].T).astype(bf16),
            "cosb": cosb,
            "sinb": sinb,
            "mask4": mask4,
            "maskb": maskb,
        })
    return in_maps


def run(inputs, trace=False):
    """Compile once, run on 8 cores, host-sum partials."""
    import sys
    if "/opt/trn_rl_repo" not in sys.path:
        sys.path.insert(0, "/opt/trn_rl_repo")
    from concourse.bass_utils import run_bass_kernel_spmd

    if "nc" not in _nc_cache:
        _nc_cache["nc"] = _build_nc()
    nc = _nc_cache["nc"]

    in_maps = make_core_inputs(**inputs)
    res = run_bass_kernel_spmd(nc, in_maps, core_ids=list(range(NCORES)),
                               trace=trace)
    out = np.zeros((T, C), np.float64)
    for r in res.results:
        out += r["opart"].astype(np.float64)
    return out.astype(np.float32).reshape(1, T, C), res


def kernel(**inputs):
    import sys
    if "/opt/trn_rl_repo" not in sys.path:
        sys.path.insert(0, "/opt/trn_rl_repo")
    out, _ = run(inputs)
    return out


# revision 11
# speedup vs baseline: 1.0374x; 1.0163x over previous
"""Causal self-attention with RoPE, tensor-parallel over heads on 8 trn2 cores.

Reference computation (B=1, T=4096, C=1024, h=16, d=64, fp32):
    q/k/v = x @ W{q,k,v}^T ; rope(q), rope(k) ; causal softmax(q k^T / 8) v ; @ Wo^T

Sharding: 2 heads per core (tensor parallel). Each core reads the full x
(transposed + bf16 on host) and its slice of Wq/Wk/Wv (column-parallel) and
Wo (row-parallel). Cores emit partial o-projections; the host sums them.

Device-side layout choices (v2):
  - qT/kT [dhead(=128 both heads) x T] with the head dim de-interleaved
    (rope real parts in partitions 0-31 / 64-95, imag in 32-63 / 96-127) so
    rope's pair-swap is a partition-block swap done by 4 small DMAs.
  - scores are computed transposed: sT[j, i] = sum_d kT[d,j] qT[d,i].
  - exp runs on ACT (hw Exp) or DVE (bf16 Schraudolph: bitcast(int16(
    x*A + B)), ~1% softmax-weight ripple that largely cancels between the
    softmax numerator and denominator); diagonal blocks fold the causal
    mask into a DVE scalar_tensor_tensor with a +B / -20000 tile (masked
    lanes land at ~1e-37 ~ 0), or run ACT exp + 0/1 mask multiply,
    restricted to the valid column range.  A static schedule balances the
    two engines.
  - att@v is computed with att as the *stationary* operand ([128k x 128q]
    blocks) and the ones-augmented v [128k x 65] moving, 65 PE cycles per
    block; the 4 query sub-blocks of a head accumulate into one psum bank
    as a single lazily-zeroed accumulation group (row 64 = softmax
    denominator); fully-masked diagonal sub-blocks are skipped.
  - normalize: per-query reciprocal of the denominators + scaled
    psum->sbuf copies per (qs, head); one batched XBAR dma-transpose per
    window rebuilds yT [dd x tok] for the o-projection (bf16), whose
    output rows go out in 256-row batched DMAs.
"""

import numpy as np
import ml_dtypes

bf16 = ml_dtypes.bfloat16

T, C, H, D = 4096, 1024, 16, 64
NCORES = 8
HPC = H // NCORES          # heads per core
DD = HPC * D               # per-core qkv features (=128)
P = 128

A0 = 128.0 / np.log(2.0)   # Schraudolph bf16: e^x ~ bitcast(int16(x*A0 + B0))
B0 = 127.0 * 128.0 - 7.0
SCALE = 0.125
MASKED = -20000.0

_nc_cache = {}


def _exp_schedule(nw):
    """(iw, jc) -> 'a' | 'd' (interior) | 'am' | 'dm' (diagonal)."""
    import os
    sched = {}
    k = dk = 0
    pat = tuple(os.environ.get("EXP_PAT", "adad"))
    dpat = tuple(c + "m" for c in os.environ.get("DIAG_PAT", "aada"))
    pat_l = tuple(os.environ.get("EXP_PAT_L", os.environ.get(
        "EXP_PAT", "adad")))
    dpat_l = tuple(c + "m" for c in os.environ.get(
        "DIAG_PAT_L", "aaaa"))
    if os.environ.get("EXP_ALL") == "a":
        pat, dpat = ("a",), ("am",)
    elif os.environ.get("EXP_ALL") == "d":
        pat, dpat = ("d",), ("dm",)
    for iw in range(nw):
        njc = 4 * iw + 4
        p, dp = (pat, dpat) if iw < int(os.environ.get("PSPLIT", str(nw - 3))) else (pat_l, dpat_l)
        if os.environ.get("KRESET") == "1":
            k = 0
        for jc in range(njc):
            if jc >= 4 * iw:
                sched[(iw, jc)] = dp[(jc - 4 * iw) % len(dp)]
                dk += 1
            else:
                sched[(iw, jc)] = p[k % len(p)]
                k += 1
    return sched


def _build_nc(t=T):
    import concourse.bass as bass
    import concourse.tile as tile
    import concourse.mybir as mybir
    from concourse import bacc

    f32 = mybir.dt.float32
    b16 = mybir.dt.bfloat16
    i16 = mybir.dt.int16
    MUL = mybir.AluOpType.mult
    ADD = mybir.AluOpType.add
    EXP = mybir.ActivationFunctionType.Exp
    CPY = mybir.ActivationFunctionType.Copy

    nt = t // 512            # qkv t-chunks
    nw = t // 512            # attention query windows
    njb = t // P             # key blocks
    AS = SCALE * A0

    sched = _exp_schedule(nw)

    nc = bacc.Bacc("TRN2")

    xt_d = nc.dram_tensor("xt", [C, t], b16, kind="ExternalInput")
    # weights host-prepacked to [P, C//P, DD]: one contiguous 2KB run per
    # partition (full-rate DMA descriptors)
    wq_d = nc.dram_tensor("wq", [P, C // P, DD], b16, kind="ExternalInput")
    wk_d = nc.dram_tensor("wk", [P, C // P, DD], b16, kind="ExternalInput")
    wv_d = nc.dram_tensor("wv", [P, C // P, DD], b16, kind="ExternalInput")
    wo_d = nc.dram_tensor("wo", [DD, C], b16, kind="ExternalInput")
    cos_d = nc.dram_tensor("cosb", [P, t], b16, kind="ExternalInput")
    sin_d = nc.dram_tensor("sinb", [P, t], b16, kind="ExternalInput")
    msk_d = nc.dram_tensor("mask4", [P, 4, 512], b16, kind="ExternalInput")
    mskb_d = nc.dram_tensor("maskb", [P, 4, 512], f32, kind="ExternalInput")
    out_d = nc.dram_tensor("opart", [t, C], b16, kind="ExternalOutput")

    with tile.TileContext(nc) as tc:
        with (
            tc.tile_pool(name="const", bufs=1) as constp,
            tc.tile_pool(name="xload", bufs=int(__import__("os").environ.get("XLB", "3"))) as xload,
            tc.tile_pool(name="rope", bufs=int(__import__("os").environ.get("RPB", "5"))) as ropep,
            tc.tile_pool(name="att", bufs=4) as attp,
            tc.tile_pool(name="small", bufs=4) as smallp,
            tc.tile_pool(name="ps", bufs=1, space="PSUM") as psp,
        ):
            # ---- constants / persistent tensors. DMA issue order chosen so
            # the first qkv matmuls (wq + x chunk 0) start ASAP.
            xt_pre = {}

            def load_xt(tch):
                # two half-loads so small latency-critical DMAs (rope swaps,
                # XBAR transposes) can slip between them on the DMA engines
                tsl = slice(tch * 512, (tch + 1) * 512)
                xt = xload.tile([P, C // P, 512], b16, name="xt", tag="xt")
                xv = xt_d[:].rearrange("(co p) t -> p co t", p=P)[:, :, tsl]
                import os as _o8
                if _o8.environ.get("XTSPLIT", "0") == "1":
                    nc.sync.dma_start(xt[:, 0:4], xv[:, 0:4])
                    nc.sync.dma_start(xt[:, 4:8], xv[:, 4:8])
                else:
                    nc.sync.dma_start(xt, xv)
                xt_pre[tch] = xt

            wq_sb = constp.tile([P, C // P, DD], b16)
            nc.sync.dma_start(wq_sb, wq_d[:])
            # x chunk 0 in four slices so the first qkv matmuls start sooner
            xt0 = xload.tile([P, C // P, 512], b16, name="xt0")
            xt_view = xt_d[:].rearrange("(co p) t -> p co t", p=P)
            for c4 in range(4):
                nc.sync.dma_start(xt0[:, 2 * c4:2 * c4 + 2],
                                  xt_view[:, 2 * c4:2 * c4 + 2, 0:512])
            xt_pre[0] = xt0
            wk_sb = constp.tile([P, C // P, DD], b16)
            nc.sync.dma_start(wk_sb, wk_d[:])
            load_xt(1)
            wv_sb = constp.tile([P, C // P, DD], b16)
            nc.sync.dma_start(wv_sb, wv_d[:])
            # rope tables: first two chunks' columns now, tails late
            cos_sb = constp.tile([P, t], b16)
            nc.sync.dma_start(cos_sb[:, 0:1024], cos_d[:, 0:1024])
            sin_sb = constp.tile([P, t], b16)
            nc.sync.dma_start(sin_sb[:, 0:1024], sin_d[:, 0:1024])
            # the first window's diagonal needs maskb[0] immediately
            mskb_sb = constp.tile([P, 4, 512], f32)
            nc.sync.dma_start(mskb_sb[:, 0:1], mskb_d[:, 0:1])
            msk_sb = constp.tile([P, 4, 512], b16)
            nc.sync.dma_start(msk_sb[:, 0:1], msk_d[:, 0:1])
            wo_sb = constp.tile([DD, C], b16)

            def late_consts():
                nc.sync.dma_start(mskb_sb[:, 1:4], mskb_d[:, 1:4])
                nc.sync.dma_start(msk_sb[:, 1:4], msk_d[:, 1:4])
                if t > 1024:
                    nc.sync.dma_start(cos_sb[:, 1024:], cos_d[:, 1024:])
                    nc.sync.dma_start(sin_sb[:, 1024:], sin_d[:, 1024:])
                nc.sync.dma_start(wo_sb, wo_d[:])

            qT = constp.tile([P, t], b16)   # rope'd q, both heads
            kT = constp.tile([P, t], b16)
            # v in natural layout per 128-block, +ones cols at 64 and 129
            vaug = constp.tile([P, njb, 2 * D + 2], b16)
            nc.vector.memset(vaug[:, :, D], 1.0)
            nc.vector.memset(vaug[:, :, 2 * D + 1], 1.0)

            # PSUM budget (8 banks): pss2 4KB x3 (scores, triple-buffered;
            # qkv and o-proj psums borrow half-slots from the same rotation)
            # + psy0/psy1 2KB x1 (att@v + denominator accumulators).
            def scratch():
                return psp.tile([P, 512], f32, tag="scratch", bufs=2,
                                name="scr")

            ob_k = {"k": 0}

            def drain_copy(dst, src_):
                """psum->sbuf copy alternating ACT/DVE."""
                ob_k["k"] += 1
                import os as _o4
                pat4 = _o4.environ.get("DRAIN", "ad")
                if pat4[ob_k["k"] % len(pat4)] == "a":
                    nc.scalar.copy(dst, src_)
                else:
                    nc.vector.tensor_copy(dst, src_)

            def qkv_parts(tch, cp):
                """Six micro-closures (~0.85us of PE each) spread through an
                attention window's PE slack. cp = engine for the psum->sbuf
                copies (ACT before the windows saturate it, DVE after)."""
                tsl = slice(tch * 512, (tch + 1) * 512)
                st = {}
                import os as _o3
                reng = nc.gpsimd if (
                    _o3.environ.get("GPSROPE", "1") == "1" and tch >= int(_o3.environ.get("GPSROPE_MIN", "2"))) \
                    else nc.vector

                def copy(dst, src_):
                    if cp is nc.scalar:
                        nc.scalar.copy(dst, src_)
                    else:
                        cp.tensor_copy(dst, src_)

                def mm_half(name, w_sb, half):
                    if name == "q" and half == 0:
                        st["xt"] = xt_pre.pop(tch)
                        st["q"] = scratch()
                    if name == "k" and half == 0:
                        if tch + 2 < nt and tch + 2 not in xt_pre:
                            load_xt(tch + 2)
                    if half == 0 and name != "q":
                        st[name] = scratch()
                    ps = st[name]
                    for ci in range(4 * half, 4 * half + 4):
                        nc.tensor.matmul(
                            ps, w_sb[:, ci], st["xt"][:, ci],
                            start=(ci == 0), stop=(ci == C // P - 1),
                        )

                def q1():
                    mm_half("q", wq_sb, 0)

                def q2():
                    mm_half("q", wq_sb, 1)
                    # q and k share one [128, 2, 512] tile; the rope
                    # pair-swap is partition-block DMAs issued right after
                    # each tensor's psum copy so the swap latency overlaps
                    # the other tensor's matmuls
                    st["qf2"] = ropep.tile([P, 2, 512], b16, tag="qf2",
                                           name="qf2")
                    st["sw2"] = ropep.tile([P, 2, 512], b16, tag="sw2",
                                           name="sw2")
                    copy(st["qf2"][:, 0], st["q"])
                    import os as _o2
                    if _o2.environ.get("EARLYSWAP") == "1":
                        qf2, sw2 = st["qf2"], st["sw2"]
                        nc.sync.dma_start(sw2[0:32, 0], qf2[32:64, 0])
                        nc.sync.dma_start(sw2[32:64, 0], qf2[0:32, 0])
                        nc.sync.dma_start(sw2[64:96, 0], qf2[96:128, 0])
                        nc.sync.dma_start(sw2[96:128, 0], qf2[64:96, 0])

                def k1():
                    mm_half("k", wk_sb, 0)

                def k2():
                    mm_half("k", wk_sb, 1)
                    qf2 = st["qf2"]
                    sw2 = st["sw2"]
                    copy(qf2[:, 1], st["k"])
                    import os as _o2
                    if _o2.environ.get("EARLYSWAP") == "1":
                        nc.sync.dma_start(sw2[0:32, 1], qf2[32:64, 1])
                        nc.sync.dma_start(sw2[32:64, 1], qf2[0:32, 1])
                        nc.sync.dma_start(sw2[64:96, 1], qf2[96:128, 1])
                        nc.sync.dma_start(sw2[96:128, 1], qf2[64:96, 1])
                    else:
                        nc.sync.dma_start(sw2[0:32], qf2[32:64])
                        nc.sync.dma_start(sw2[32:64], qf2[0:32])
                        nc.sync.dma_start(sw2[64:96], qf2[96:128])
                        nc.sync.dma_start(sw2[96:128], qf2[64:96])
                    for i, name in enumerate(("q", "k")):
                        tl = ropep.tile([P, 512], b16, tag=f"t1_{name}",
                                        name="t1")
                        reng.tensor_tensor(tl, qf2[:, i],
                                           cos_sb[:, tsl], MUL)
                        st[f"t1{name}"] = tl

                def v1():
                    t2q = ropep.tile([P, 512], b16, tag="t2_q", name="t2")
                    reng.tensor_tensor(t2q, st["sw2"][:, 0],
                                       sin_sb[:, tsl], MUL)
                    reng.tensor_add(qT[:, tsl], st["t1q"], t2q)
                    mm_half("v", wv_sb, 0)

                def v2():
                    mm_half("v", wv_sb, 1)
                    t2k = ropep.tile([P, 512], b16, tag="t2_k", name="t2")
                    reng.tensor_tensor(t2k, st["sw2"][:, 1],
                                       sin_sb[:, tsl], MUL)
                    reng.tensor_add(kT[:, tsl], st["t1k"], t2k)
                    # v: psum->sbuf copy, XBAR dma-transpose to dense [t, d]
                    # blocks, two narrow gpsimd copies around the ones column
                    vt = ropep.tile([P, 512], b16, tag="vt", name="vt")
                    copy(vt, st["v"])
                    dense = ropep.tile([P, 4, P], b16, tag="vdense",
                                       name="vdense")
                    nc.sync.dma_start_transpose(dense, vt[:])
                    g0 = tch * 4
                    nc.gpsimd.tensor_copy(vaug[:, g0:g0 + 4, 0:D],
                                          dense[:, :, 0:D])
                    nc.gpsimd.tensor_copy(vaug[:, g0:g0 + 4, D + 1:2 * D + 1],
                                          dense[:, :, D:2 * D])

                return [q1, q2, k1, k2, v1, v2]

            def qkv_chunk(tch):
                for f in qkv_parts(tch, nc.scalar):
                    f()

            def oproj_fillers(iw, yT, tail=False):
                """o-projection for two 256-row blocks, one batched output
                DMA each. Stationary yT blocks come from the window's XBAR
                transpose."""
                outs = []
                for pi in range(2):
                    def one(pi=pi):
                        ob4 = attp.tile([P, 2, 2, 512], b16, tag="ob4",
                                        bufs=int(__import__("os").environ.get(
                                            "OB4B", "3")), name="ob4")
                        tb0 = 4 * iw + 2 * pi
                        for j in range(2):
                            g = 2 * pi + j
                            for mc in range(2):
                                pso = scratch()
                                nc.tensor.matmul(
                                    pso, yT[:, g],
                                    wo_sb[:, mc * 512:(mc + 1) * 512],
                                    start=True, stop=True,
                                )
                                drain_copy(ob4[:, j, mc], pso)
                            if tail:
                                tb = tb0 + j
                                dstj = out_d[tb * P:(tb + 1) * P, :]\
                                    .rearrange("p (mc i) -> p mc i", mc=2)
                                import os as _o9
                                if _o9.environ.get("TAILQ", "0") == "1" \
                                        and pi == 1 and j == 1:
                                    for mc2 in range(2):
                                        for qq in range(2):
                                            qsl = slice(qq * 256,
                                                        qq * 256 + 256)
                                            nc.sync.dma_start(
                                                dstj[:, mc2, qsl],
                                                ob4[:, j, mc2, qsl])
                                elif _o9.environ.get("TAILMC", "1") == "1":
                                    for mc2 in range(2):
                                        nc.sync.dma_start(
                                            dstj[:, mc2], ob4[:, j, mc2])
                                else:
                                    nc.sync.dma_start(dstj, ob4[:, j])
                        if not tail:
                            dst = out_d[tb0 * P:(tb0 + 2) * P, :].rearrange(
                                "(j p) (mc i) -> p j mc i", j=2, mc=2)
                            nc.sync.dma_start(dst, ob4)
                    outs.append(one)
                return outs

            def win512(iw, parts=(), oproj=(), last=False, head=()):
                """512-wide attention window over i in [512iw, 512iw+512),
                one-jc software-pipelined. Fillers run in the slack of the
                jc loop: qkv chunk micro-parts from jc>=1, the o-projection
                of window iw-2 from jc>=3."""
                isl = slice(iw * 512, (iw + 1) * 512)
                njc = 4 * iw + 4
                parts = list(parts)
                oproj = list(oproj)
                psy = [psp.tile([P, 4, P], f32, tag=f"psy{h}",
                                name="psy") for h in range(HPC)]

                def score_exp(jc, isl_, oiw):
                    eng = sched[(oiw, jc)]
                    m = jc - 4 * oiw if jc >= 4 * oiw else None
                    import os as _o
                    lo = m * P if (m is not None and _o.environ.get(
                        "DIAG_SCORE") == "1") else 0
                    jsl = slice(jc * P, (jc + 1) * P)
                    ps2 = psp.tile([P, 2, 512], f32, tag="pss2", bufs=2,
                                   name="ps2")
                    for h in range(HPC):
                        hb = D * h
                        nc.tensor.matmul(
                            ps2[:, h, lo:],
                            kT[hb:hb + D, jsl],
                            qT[hb:hb + D, isl_.start + lo:isl_.stop],
                            start=True, stop=True)
                    att2 = attp.tile([P, 2, 512], b16, tag="att2",
                                     name="att2", bufs=int(__import__("os").environ.get("A2B", "14")))
                    if m is None:
                        if _o.environ.get("EXPSPLIT") == "1":
                            nc.scalar.activation(att2[:, 0], ps2[:, 0], EXP,
                                                 scale=SCALE)
                            nc.vector.tensor_scalar(
                                att2[:, 1].bitcast(i16), ps2[:, 1],
                                AS, B0, MUL, ADD)
                        elif _o.environ.get("EXPSPLIT") == "2":
                            if eng == "a":
                                nc.scalar.activation(att2[:, 0], ps2[:, 0],
                                                     EXP, scale=SCALE)
                                nc.scalar.activation(att2[:, 1], ps2[:, 1],
                                                     EXP, scale=SCALE)
                            else:
                                for hh in range(2):
                                    nc.vector.tensor_scalar(
                                        att2[:, hh].bitcast(i16),
                                        ps2[:, hh], AS, B0, MUL, ADD)
                        elif eng == "a":
                            nc.scalar.activation(att2, ps2, EXP, scale=SCALE)
                        else:
                            nc.vector.tensor_scalar(att2[:].bitcast(i16),
                                                    ps2, AS, B0, MUL, ADD)
                    else:
                        lo = m * P
                        if eng == "am":
                            nc.scalar.activation(att2[:, :, lo:],
                                                 ps2[:, :, lo:], EXP,
                                                 scale=SCALE)
                            for h in range(HPC):
                                nc.vector.tensor_tensor(
                                    att2[:, h, lo:], att2[:, h, lo:],
                                    msk_sb[:, m, lo:], MUL)
                        else:
                            for h in range(HPC):
                                nc.vector.scalar_tensor_tensor(
                                    att2[:, h, lo:].bitcast(i16),
                                    ps2[:, h, lo:], AS,
                                    mskb_sb[:, m, lo:], MUL, ADD)
                    return att2, m

                def emit_attv(p):
                    jc0, a2, m0 = p
                    for h in range(HPC):
                        va = vaug[:, jc0, 65 * h:65 * h + 65]
                        for qs in range(4):
                            if m0 is not None and qs < m0:
                                continue
                            nc.tensor.matmul(
                                psy[h][:, qs, 0:65],
                                a2[:, h, qs * P:(qs + 1) * P], va,
                                start=(jc0 == 0 and qs == 0),
                                stop=(jc0 == njc - 1 and qs == 3))

                pend = list(head)
                for jc in range(len(head), njc):
                    att2, m = score_exp(jc, isl, iw)
                    if len(pend) == int(__import__("os").environ.get(
                            "PEND", "2")):
                        emit_attv(pend.pop(0))
                    pend.append((jc, att2, m))
                    import os as _oA
                    if jc < njc - 3:
                        if jc >= int(_oA.environ.get("PJC", "1")) and parts:
                            parts.pop(0)()
                        elif jc >= int(_oA.environ.get("OJC", "3")) \
                                and oproj and (
                                _oA.environ.get("OEVERY") == "1"
                                or jc % 2 == 1
                                or 3 * len(oproj) > njc - 3 - jc):
                            oproj.pop(0)()
                nxt = []
                import os as _oB
                if _oB.environ.get("W0NXT", "0") == "1" and iw == 0:
                    for f in parts:   # chunk-1 rope must precede win1 scores
                        f()
                    parts = []
                if not last and (iw > 0 or _oB.environ.get(
                        "W0NXT", "0") == "1"):
                    # pre-compute the next window's first score/exp blocks
                    # so PE has work while this window's tail drains
                    isl_n = slice((iw + 1) * 512, (iw + 2) * 512)
                    import os as _o6
                    _nxt = int(_o6.environ.get("NXT", "3"))
                    if iw <= int(_o6.environ.get("NXT4MAX", "-1")):
                        _nxt = 4
                    if iw >= int(_o6.environ.get("NXTLMIN", "4")):
                        _nxt = int(_o6.environ.get("NXTL", "6"))
                    for jc_n in range(_nxt):
                        a2n, mn = score_exp(jc_n, isl_n, iw + 1)
                        nxt.append((jc_n, a2n, mn))
                for p in pend:
                    emit_attv(p)
                recs = []
                for h in range(HPC):
                    rec = smallp.tile([P, 4], f32, tag=f"rec{h}", bufs=int(__import__("os").environ.get("RECB", "3")),
                                      name="rec")
                    nc.vector.reciprocal(rec, psy[h][:, :, 64])
                    recs.append(rec)
                yT = ropep.tile([P, 4, P], b16, tag="yt", bufs=int(__import__("os").environ.get("YTB", "3")), name="yT")

                def norm_half(q0):
                    def f():
                        ysb = attp.tile([P, 2, P], b16, tag=f"ysb{q0}",
                                        bufs=int(__import__("os").environ.get("YSBB", "2")), name="ysb")
                        for qs in (q0, q0 + 1):
                            for h in range(HPC):
                                dst = ysb[:, qs - q0, 64 * h:64 * h + 64]
                                srcp = psy[h][:, qs, 0:64]
                                import os as _o7
                                nrm = _o7.environ.get(
                                    "NORMENG", "alt" if last else "a")
                                if nrm == "a" or (nrm == "alt"
                                                  and (qs + h) % 2 == 0):
                                    nc.scalar.activation(
                                        dst, srcp, CPY,
                                        scale=recs[h][:, qs:qs + 1])
                                else:
                                    nc.vector.tensor_scalar(
                                        dst, srcp, recs[h][:, qs:qs + 1],
                                        None, MUL)
                        nc.sync.dma_start_transpose(
                            yT[:, q0:q0 + 2, :],
                            ysb[:].rearrange("p q d -> p (q d)"))
                    return f

                import os
                if os.environ.get("OLD_NORM") == "1":
                    ysb = attp.tile([P, 4, P], b16, tag="ysb0", bufs=2,
                                    name="ysb")
                    for qs in range(4):
                        for h in range(HPC):
                            dst = ysb[:, qs, 64 * h:64 * h + 64]
                            srcp = psy[h][:, qs, 0:64]
                            if (qs + h) % 2 == 0:
                                nc.scalar.activation(
                                    dst, srcp, CPY,
                                    scale=recs[h][:, qs:qs + 1])
                            else:
                                nc.vector.tensor_scalar(
                                    dst, srcp, recs[h][:, qs:qs + 1],
                                    None, MUL)
                    nc.sync.dma_start_transpose(
                        yT, ysb[:].rearrange("p q d -> p (q d)"))
                    norms = []
                else:
                    norms = [norm_half(0), norm_half(2)]
                if norms and (last or os.environ.get("NO_DEFER_NORM") == "1"):
                    for f in norms:
                        f()
                    norms = []
                for f in parts + oproj:   # leftovers
                    f()
                return oproj_fillers(iw, yT, tail=last), nxt, norms

            qkv_chunk(0)
            c1 = qkv_parts(1, nc.scalar)
            for f in c1[:4]:
                f()
            late_consts()

            pend_ops = {}
            hd = ()
            norms = []
            for iw in range(nw):
                if iw == 0:
                    parts = c1[4:] + (qkv_parts(2, nc.scalar)
                                      if nt > 2 else [])
                else:
                    tch = iw + 2
                    import os as _o5
                    _cpm = int(_o5.environ.get("CPMAX", "7"))
                    parts = qkv_parts(
                        tch, nc.scalar if tch <= _cpm else nc.vector) \
                        if tch < nt else []
                if iw == nw - 2:
                    oproj = pend_ops.pop(iw - 2, []) + pend_ops.pop(iw - 1,
                                                                    [])
                elif iw == nw - 1:
                    oproj = pend_ops.pop(iw - 1, [])
                else:
                    oproj = pend_ops.pop(iw - 2, [])
                ops, hd, norms = win512(iw, parts=norms + parts,
                                        oproj=oproj,
                                        last=(iw == nw - 1), head=hd)
                pend_ops[iw] = ops
            for f in pend_ops.pop(nw - 1):
                f()

    nc.compile()
    return nc


def _perm_deinterleave():
    """Row permutation for Wq/Wk: per head, even rows then odd rows."""
    perm = []
    for h in range(H):
        base = h * D
        perm += [base + 2 * k for k in range(D // 2)]
        perm += [base + 2 * k + 1 for k in range(D // 2)]
    return np.array(perm)


def make_core_inputs(x, freqs_cos, freqs_sin, Wq, Wk, Wv, Wo, t=T):
    """Host-side sharding/layout prep. Returns per-core input dicts."""
    x = np.asarray(x, np.float32).reshape(t, C)
    fc = np.asarray(freqs_cos, np.float32)
    fs = np.asarray(freqs_sin, np.float32)
    Wq = np.asarray(Wq, np.float32)
    Wk = np.asarray(Wk, np.float32)
    Wv = np.asarray(Wv, np.float32)
    Wo = np.asarray(Wo, np.float32)

    xt = np.ascontiguousarray(x.T).astype(bf16)                  # [C, t]
    perm = _perm_deinterleave()
    Wq_p, Wk_p = Wq[perm], Wk[perm]

    # rope factor tables in the de-interleaved [dd, t] layout
    kidx = np.arange(P) % 32
    sgn = np.where((np.arange(P) // 32) % 2 == 0, -1.0, 1.0).astype(np.float32)
    cosb = fc.T[kidx].astype(bf16)                               # [128, t]
    sinb = (fs.T[kidx] * sgn[:, None]).astype(bf16)

    # diagonal-tile causal masks: valid iff 128*m + j <= i
    jj = np.arange(P)[:, None, None]
    mm = np.arange(4)[None, :, None]
    ii = np.arange(512)[None, None, :]
    valid = (P * mm + jj) <= ii
    mask4 = valid.astype(bf16)
    maskb = np.where(valid, np.float32(B0), np.float32(MASKED))

    def pack_w(w):
        # [C, DD] -> [P, C//P, DD]: one contiguous 2KB run per partition
        return np.ascontiguousarray(
            w.T.reshape(C // P, P, DD).transpose(1, 0, 2)).astype(bf16)

    in_maps = []
    for c in range(NCORES):
        rows = slice(c * DD, (c + 1) * DD)
        in_maps.append({
            "xt": xt,
            "wq": pack_w(Wq_p[rows]),
            "wk": pack_w(Wk_p[rows]),
            "wv": pack_w(Wv[rows]),
            "wo": np.ascontiguousarray(Wo[:, rows].T).astype(bf16),
            "cosb": cosb,
            "sinb": sinb,
            "mask4": mask4,
            "maskb": maskb,
        })
    return in_maps


def run(inputs, trace=False):
    """Compile once, run on 8 cores, host-sum partials."""
    import sys
    if "/opt/trn_rl_repo" not in sys.path:
        sys.path.insert(0, "/opt/trn_rl_repo")
    from concourse.bass_utils import run_bass_kernel_spmd

    if "nc" not in _nc_cache:
        _nc_cache["nc"] = _build_nc()
    nc = _nc_cache["nc"]

    in_maps = make_core_inputs(**inputs)
    res = run_bass_kernel_spmd(nc, in_maps, core_ids=list(range(NCORES)),
                               trace=trace)
    out = np.zeros((T, C), np.float64)
    for r in res.results:
        out += r["opart"].astype(np.float64)
    return out.astype(np.float32).reshape(1, T, C), res


def kernel(**inputs):
    import sys
    if "/opt/trn_rl_repo" not in sys.path:
        sys.path.insert(0, "/opt/trn_rl_repo")
    out, _ = run(inputs)
    return out
